# revision 40
# baseline (speedup 1.0000x reference)
"""AdaptiveAdjacency Bass kernel for 8 TRN2 NeuronCores.

Reference computation per batch b (N=1024 nodes, H=24 hidden):
    Z   = relu(xt @ W + b)                    (N, H)
    A   = Z @ Z.T                             (N, N)  -- symmetric!
    A   = 0.5*(softmax(A, -1) + softmax(A, -2)) + I
    deg = A.sum(-1);  out = A * deg^-1/2 [row] * deg^-1/2 [col]

Math used here (exploiting symmetry of A_raw):
    E = exp(A_raw - 40)            (shift is softmax-invariant; A_raw max ~54)
    softmax(A,-2) == softmax(A,-1).T, so with r = 1/rowsum(E):
        A_sym[n,m] = E[n,m] * 0.5*(r[n]+r[m]) + I
    Fold "+I" into E:  E' = E + diag(rowsum)  =>  out = E' * C with
        C[n,m] = u[n]v[m] + v[n]u[m],  u = 0.5*r*ds,  v = ds,
        ds = degree^-1/2, degree = 1 + 0.5*colsum(r[n]*E'[n,m])
    C is rank-2 -> one K=2 matmul per output tile; single elementwise
    multiply per output element (the only full-size DVE pass).

Sharding: data-parallel over B=32 across 8 cores (4 batches each);
W/b replicated. Host pre-transposes xt to (B, F, N) fp16 so the tiny
Linear runs as a natural PE matmul (contraction over F on partitions).
"""

import numpy as np

import concourse.bass as bass
import concourse.tile as tile
from concourse import bacc, mybir
from concourse.masks import make_identity
from concourse.bass_utils import run_bass_kernel_spmd

B_FULL = 32
B_LOC = 4  # batches per core
N = 1024
F = 64
H = 24
NT = N // 128  # 8 row tiles
CK = 512  # matmul free chunk (one PSUM bank)
NCK = N // CK
KSHIFT = -40.0  # softmax shift (global constant: softmax-invariant)
N_CORES = 8

f32 = mybir.dt.float32
bf16 = mybir.dt.bfloat16
fp16 = mybir.dt.float16
AF = mybir.ActivationFunctionType
ALU = mybir.AluOpType


_TABLES_PATCHED = False


def _force_single_act_table_set():
    """All activation funcs used here (Exp, Ln, Relu, Copy/Identity) live in
    the natural_log_exp_and_others set. bacc's table-load inserter picks the
    first set containing each function, which thrashes ~2.7us per switch
    between exp_and_others and natural_log. Strip those functions from every
    other set (indices must be preserved) so one table load covers the
    whole kernel."""
    global _TABLES_PATCHED
    if _TABLES_PATCHED:
        return
    _TABLES_PATCHED = True
    import concourse.hw_specs as hw_specs

    orig = hw_specs.get_activation_tables
    keep = {
        AF.Exp,
        AF.Ln,
        AF.Relu,
        AF.Copy,
        AF.Identity,
        AF.Square,
        AF.Abs,
        AF.Sign,
        AF.MemsetZero,
        AF.Is_finite,
    }
    target = "natural_log_exp_and_others"

    def patched(module_arch):
        tables = orig(module_arch)
        if target not in tables:
            return tables
        out = {}
        for name, funcs in tables.items():
            out[name] = funcs if name == target else (funcs - keep)
        return out

    hw_specs.get_activation_tables = patched
    bacc.get_activation_tables = patched


def build_nc(
    repeat: int = 1, timing_trip: int | None = None, ablate: str | None = None
) -> bass.Bass:
    """timing_trip=T builds a timing variant: the whole computation runs in
    an on-device For_i loop T times, writing to internal DRAM scratch with a
    tiny external output, so real device time per iteration can be measured
    by wall-clock differencing of two trip counts (fixed host/transfer costs
    cancel; code size is constant)."""
    abl = set(ablate.split(",")) if ablate else set()
    if "nocs1bank" not in abl:
        abl.add("cs1bank")  # single-bank colsum accumulator (frees a PSUM
        # bank so the C-matmul pool triple-buffers)
    _force_single_act_table_set()
    nc = bacc.Bacc()
    xtT = nc.declare_dram_parameter("xtT", [B_LOC, F, N], fp16, isOutput=False)
    Wd = nc.declare_dram_parameter("W", [F, H], fp16, isOutput=False)
    bd = nc.declare_dram_parameter("b", [H, 1], f32, isOutput=False)
    if timing_trip is None:
        outd = nc.declare_dram_parameter("out", [B_LOC, N, N], bf16, isOutput=True)
    else:
        outd = nc.dram_tensor("oscratch", [B_LOC, N, N], bf16)
        tiny_out = nc.declare_dram_parameter("out", [2, 2], f32, isOutput=True)
    # host constants (engine APs must start at partition 0, so these cannot
    # be built with sliced memsets):
    #   cst col 0/1: per-partition scale/bias for the fused colsum Ln
    #   cuv cols 0:2 = Cu, 2:4 = Cv (f32r lhsT for the log-mix matmuls)
    cstd = nc.declare_dram_parameter("cst", [34, 4], f32, isOutput=False)
    cuvd = nc.declare_dram_parameter("cuv", [34, 4], mybir.dt.float32r, isOutput=False)


    with tile.TileContext(nc) as tc:
        with (
            tc.tile_pool(name="singles", bufs=1) as singles,
            tc.tile_pool(name="zpool", bufs=B_LOC) as zpool,
            tc.tile_pool(name="epool", bufs=6 + 2 * NT) as epool,
            tc.tile_pool(name="vpool", bufs=2) as vpool,
            tc.tile_pool(name="opool", bufs=8 if (ablate and "op8" in ablate) else 6) as opool,
            tc.tile_pool(name="apool", bufs=2, space="PSUM") as apool,
            tc.tile_pool(name="cpool", bufs=2 if (ablate and "nocs1bank" in ablate) else 3, space="PSUM") as cpool,
            tc.tile_pool(name="cspool", bufs=1, space="PSUM") as cspool,
        ):
            wsb = singles.tile([F, H], fp16)
            nc.gpsimd.dma_start(wsb[:], Wd[:, :])
            bsb = singles.tile([H, 1], f32)
            nc.gpsimd.dma_start(bsb[:], bd[:, :])
            ident = singles.tile([128, 128], bf16)
            make_identity(nc, ident[:])
            cm40 = singles.tile([128, 1], f32)
            nc.gpsimd.memset(cm40[:], KSHIFT)
            # per-partition [scale, bias] for the fused colsum Ln:
            # row 0: ln(0.5*cs0 + 1.0)   row 1: ln(1.0*cs1 + 0.0)
            cstsb = singles.tile([34, 4], f32)
            nc.gpsimd.dma_start(cstsb[:], cstd[:, :])
            cuvsb = singles.tile([34, 4], mybir.dt.float32r)
            nc.gpsimd.dma_start(cuvsb[:], cuvd[:, :])

            # ---- Z^T = relu(W^T @ xt^T + b) : [H, N] fp16, all batches
            # upfront (fills otherwise-idle engines during pipeline fill and
            # removes the Z chain from the batch-boundary critical path) ----
            zts = []
            for b in range(B_LOC):
                xtsb = zpool.tile([F, N], fp16, tag="xt")
                nc.sync.dma_start(xtsb[:], xtT[b])
                zpsum = apool.tile([H, N], f32, tag="ps")
                for j in range(NCK):
                    nc.tensor.matmul(
                        zpsum[:, j * CK : (j + 1) * CK],
                        wsb[:],
                        xtsb[:, j * CK : (j + 1) * CK],
                        start=True,
                        stop=True,
                    )
                zt = zpool.tile([H, N], fp16, tag="zt")
                # relu on DVE (ACT is the bottleneck engine): (Zpre + b) max 0
                nc.vector.tensor_scalar(
                    zt[:], zpsum[:], bsb[:], 0.0, ALU.add, ALU.max
                )
                # replicas at partitions 32/64/96: A_raw matmuls spread over
                # all four PE row groups (K=24 fits a 32-row group) so up to
                # four streams run concurrently in the array
                zreps = [zt]
                for g in (32, 64, 96):
                    ztg = zpool.tile([g + H, N], fp16, tag=f"zt{g}")
                    nc.sync.dma_start(ztg[g : g + H, :], zt[:])
                    zreps.append(ztg)
                zts.append(zreps)
            # r_buf col 1 stays 1.0 forever; col 0 is overwritten by the
            # recips each batch. One tile + one memset for the whole kernel.
            r_buf = singles.tile([128, NT, 2], bf16)
            nc.gpsimd.memset(r_buf[:], 1.0)

            def stats_tile(b, i, ztpair, rowsums, r_buf, cs, e_tiles):
                """A_raw matmul, exp(+rowsum), r_i, colsum accumulate.

                The colsum runs on PLAIN E (not E' = E + diag(rowsum)):
                degree = 1.5 + 0.5*colsum(r*E) and cs1 = colsum(E) = rowsum
                (by symmetry), so the diag fix stays off this critical
                chain -- it is emitted later, anywhere before the c-phase.

                The (chunk, tile-parity) pair selects one of the four PE row
                groups via Z replicas at partitions 0/32/64/96, so the two
                chunk matmuls of a tile AND adjacent tiles all overlap in
                the systolic array."""
                zreps = ztpair
                apsum = apool.tile([128, N], f32, tag="ps")
                for j in range(NCK):
                    g = 2 * (i % 2) + j  # 0..3
                    z = zreps[g]
                    base = (32 * g, 32 * g + H)
                    zs = z[base[0] : base[1], :] if g else z[:, :]
                    nc.tensor.matmul(
                        apsum[:, j * CK : (j + 1) * CK],
                        zs[:, i * 128 : (i + 1) * 128],
                        zs[:, j * CK : (j + 1) * CK],
                        start=True,
                        stop=True,
                        tile_position=(32 * g, 0),
                    )
                et = epool.tile([128, N], bf16, tag="E")
                nc.scalar.activation(
                    et[:],
                    apsum[:],
                    AF.Exp,
                    bias=cm40[:],
                    accum_out=rowsums[:, i, :],
                )
                if i % 2 == 1:
                    # one paired reciprocal per two tiles: a [128,1] recip
                    # costs ~455ns of DVE but [128,2] only ~150 (fixed
                    # dispatch dominates); the pair forces CSLAG=2
                    with nc.allow_low_precision("bf16 r for colsum lhsT"):
                        nc.vector.reciprocal(
                            r_buf[:, i - 1 : i + 1, 0:1], rowsums[:, i - 1 : i + 1, :]
                        )
                e_tiles.append(et)

            def colsum_tile(i, r_buf, cs, e_tiles):
                """Accumulate cs += [r_i, 1]^T @ E_i. Issued CSLAG tiles
                behind the exp so the in-order PE never stalls on the
                exp/recip semaphores. cs1bank layout: column-half j lands at
                partition rows 32j:32j+2 (tile_position col group j), so the
                accumulator fits one PSUM bank and the two chunks stream in
                different array column groups."""
                if "nocolsum" in abl:
                    return
                lhs = ident[:, 0:2] if "csnor" in abl else r_buf[:, i, :]
                for j in range(NCK):
                    if "cs1bank" in abl:
                        out_ap = cs[32 * j : 32 * j + 2, :]
                        tp = (0, 32 * j)
                    else:
                        out_ap = cs[:, j * CK : (j + 1) * CK]
                        tp = None
                    nc.tensor.matmul(
                        out_ap,
                        lhs,
                        e_tiles[i][:, j * CK : (j + 1) * CK],
                        start=(i == 0),
                        stop=(i == NT - 1),
                        skip_group_check=True,
                        tile_position=tp,
                    )

            def batch_tail(b, cs):
                """degree -> ds; u, v vectors (free layout).
                degree = 1 + 0.5*cs0 ; v = ds = exp(-0.5*ln(degree))
                u = 0.5*r*ds = exp(-ln(cs1) - 0.5*ln(degree))  (cs1 = 2*rowsum)
                Engine ops must be lane-aligned (partition base 0), so the
                log-domain row mixing runs on the PE (K=2 f32r matmuls
                against a tiny constant lhsT), never across partitions."""
                uv = vpool.tile([2, N], bf16, tag="uv")  # [u; v] (lhsT source)
                vu = vpool.tile([2, N], bf16, tag="vu")  # [v; u] (rhs source)
                if "cs1bank" in abl:
                    lls = vpool.tile([34, CK], mybir.dt.float32r, tag="lls")
                    nc.scalar.activation(
                        lls[:], cs[:, :], AF.Ln, bias=cstsb[:, 1:2], scale=cstsb[:, 0:1]
                    )
                else:
                    lls = vpool.tile([2, N], mybir.dt.float32r, tag="lls")
                    nc.scalar.activation(
                        lls[:],
                        cs[:, :],
                        AF.Ln,
                        bias=cstsb[0:2, 1:2],
                        scale=cstsb[0:2, 0:1],
                    )
                # log-mix matmuls write cpool slots (idle until the c-phase),
                # so the cs accumulator frees right after the Ln and the next
                # batch's colsum can begin during this batch's tail
                for coeff, dst in ((0, uv), (2, vu)):
                    lmix = apool.tile([2, N], f32, tag="ps")
                    for j in range(NCK):
                        if "cs1bank" in abl:
                            lhs_ap = cuvsb[32 * j : 32 * j + 2, coeff : coeff + 2]
                            rhs_ap = lls[32 * j : 32 * j + 2, :]
                            tp = (32 * j, 0)
                        else:
                            lhs_ap = cuvsb[0:2, coeff : coeff + 2]
                            rhs_ap = lls[:, j * CK : (j + 1) * CK]
                            tp = None
                        nc.tensor.matmul(
                            lmix[:, j * CK : (j + 1) * CK],
                            lhs_ap,
                            rhs_ap,
                            start=True,
                            stop=True,
                            tile_position=tp,
                        )
                    # bias ln(0.5) on the u row only (cs1 is rowsum, not
                    # 2*rowsum, so u = exp(mix + ln 0.5))
                    nc.scalar.activation(
                        dst[:],
                        lmix[:],
                        AF.Exp,
                        bias=cstsb[0:2, 2 + coeff // 2 : 3 + coeff // 2],
                    )
                # partition-64 replicas so odd c-tiles run in PE row group 64
                # (lhsT and rhs must share the same base partition). HWDGE
                # (sync engine) so no Pool/Q7 descriptor generation.
                uv64 = vpool.tile([66, N], bf16, tag="uv64")
                nc.scalar.dma_start(uv64[64:66, :], uv[:])
                vu64 = vpool.tile([66, N], bf16, tag="vu64")
                nc.scalar.dma_start(vu64[64:66, :], vu[:])
                return uv, vu, uv64, vu64

            def diag_patch(i, rowsums, e_tiles):
                """E'[n,n] = E[n,n] + rowsum[n] (folds "+I" into the final
                multiply; one cheap TS+TT pair per tile, ~420ns of DVE).
                Measured faster on DVE than Pool with the triple-buffered
                cpool (Pool ops hold the shared DVE/Pool SBUF port).
                Ordered after the colsum's plain-E read by Tile's WAR
                tracking; needed before the c-phase."""
                et = e_tiles[i]
                eng = nc.gpsimd if "diagpool" in abl else nc.vector
                dtmp = vpool.tile([128, 128], bf16, tag="dtmp")
                eng.tensor_scalar_mul(dtmp[:], ident[:], rowsums[:, i, :])
                eng.tensor_tensor(
                    et[:, i * 128 : (i + 1) * 128],
                    et[:, i * 128 : (i + 1) * 128],
                    dtmp[:],
                    ALU.add,
                )

            def c_tile(b, i, uv, vu, uv64, vu64, e_tiles, last_batch):
                """C = u v^T + v u^T (K=2 matmul), out = E' * C, DMA out."""
                osb = opool.tile([128, N], bf16, tag="o")
                if i % 2 == 0 or "norep" in abl:
                    uvs, vus = uv, vu
                else:
                    uvs, vus = uv64[64:66, :], vu64[64:66, :]
                for j in range(NCK):
                    cps = cpool.tile([128, CK], f32, tag="c")
                    nc.tensor.matmul(
                        cps[:],
                        uvs[:, i * 128 : (i + 1) * 128],
                        vus[:, j * CK : (j + 1) * CK],
                        start=True,
                        stop=True,
                    )
                    if last_batch and i < 5 and "nodrainsplit" not in abl:
                        # drain phase: ACT is idle -- route the PSUM exit
                        # through an ACT Copy so the multiply runs at the
                        # 2x both-SBUF TT rate (DVE 824ns/tile vs 1408)
                        csb = opool.tile([128, CK], bf16, tag="csb")
                        nc.scalar.activation(csb[:], cps[:], AF.Copy)
                        nc.vector.tensor_tensor(
                            osb[:, j * CK : (j + 1) * CK],
                            e_tiles[i][:, j * CK : (j + 1) * CK],
                            csb[:],
                            ALU.mult,
                        )
                    else:
                        nc.vector.tensor_tensor(
                            osb[:, j * CK : (j + 1) * CK],
                            e_tiles[i][:, j * CK : (j + 1) * CK],
                            cps[:],
                            ALU.mult,
                        )
                if "nodma" not in abl:
                    nc.sync.dma_start(outd[b, i * 128 : (i + 1) * 128, :], osb[:])


            # software pipeline: batch b's stats tiles interleave with batch
            # b-1's output tiles so PE/DVE/DMA trail ACT by one phase.
            # At each batch crossing, the next batch's first two stats tiles
            # are emitted BEFORE the trailing colsums + tail (keeps ACT fed
            # while the cs/Ln chain resolves), and the previous batch's last
            # two c-tiles land inside the crossing (keeps DVE fed during the
            # tail's Ln/lmix/exp latency).
            def emit_pipeline(last_rep):
                if "oldpipe" not in abl:
                    emit_pipeline_xing(last_rep)
                    return
                prev = None
                for b in range(B_LOC):
                    rowsums = vpool.tile([128, NT, 1], f32, tag="rowsums")
                    # cs[0,m] = sum_n r[n] E[n,m] = w; cs[1,m] = rowsum[m]
                    # (cs1bank: column-half j lives at partition rows 32j:32j+2
                    #  so the accumulator fits one PSUM bank)
                    if "cs1bank" in abl:
                        cs = cspool.tile([34, CK], f32, tag="cs")
                    else:
                        cs = cspool.tile([2, N], f32, tag="cs")
                    if "nocolsum" in abl:
                        nc.vector.memset(cs[:], 1.0)
                    e_tiles = []
                    if "csburst" in abl:
                        # colsums in two contiguous bursts: the K=128
                        # full-array matmuls drain the 32-row-group streams
                        # (A_raw/C) once per burst instead of per pair
                        for i in range(NT):
                            stats_tile(b, i, zts[b], rowsums, r_buf, cs, e_tiles)
                            if i == NT - 3:
                                for t in range(NT // 2):
                                    colsum_tile(t, r_buf, cs, e_tiles)
                                    if "nodiag" not in abl:
                                        diag_patch(t, rowsums, e_tiles)
                            if prev is not None and "statsonly" not in abl:
                                c_tile(prev[0], i, *prev[1:], False)
                        for t in range(NT // 2, NT):
                            colsum_tile(t, r_buf, cs, e_tiles)
                            if "nodiag" not in abl:
                                diag_patch(t, rowsums, e_tiles)
                    else:
                        CSLAG = 2 if "cslag2" in abl else (4 if "cslag4" in abl else 3)
                        for i in range(NT):
                            stats_tile(b, i, zts[b], rowsums, r_buf, cs, e_tiles)
                            if "csfirst" in abl:
                                if i >= CSLAG:
                                    colsum_tile(i - CSLAG, r_buf, cs, e_tiles)
                                    if "nodiag" not in abl:
                                        diag_patch(i - CSLAG, rowsums, e_tiles)
                                if prev is not None and "statsonly" not in abl:
                                    c_tile(prev[0], i, *prev[1:], False)
                            else:
                                if prev is not None and "statsonly" not in abl:
                                    c_tile(prev[0], i, *prev[1:], False)
                                if i >= CSLAG:
                                    colsum_tile(i - CSLAG, r_buf, cs, e_tiles)
                                    if "nodiag" not in abl:
                                        diag_patch(i - CSLAG, rowsums, e_tiles)
                        for i in range(NT - CSLAG, NT):
                            colsum_tile(i, r_buf, cs, e_tiles)
                            if "nodiag" not in abl:
                                diag_patch(i, rowsums, e_tiles)
                    uv, vu, uv64, vu64 = batch_tail(b, cs)
                    prev = (b, uv, vu, uv64, vu64, e_tiles)
                if "statsonly" not in abl:
                    for i in range(NT):
                        c_tile(prev[0], i, *prev[1:], last_rep)

            def emit_pipeline_xing(last_rep):
                PRE = 2  # next-batch stats tiles pre-emitted at the crossing
                CSLAG = 3
                states = {}

                def new_state(b):
                    rowsums = vpool.tile([128, NT, 1], f32, tag="rowsums")
                    if "cs1bank" in abl:
                        cs = cspool.tile([34, CK], f32, tag="cs")
                    else:
                        cs = cspool.tile([2, N], f32, tag="cs")
                    if "nocolsum" in abl:
                        nc.vector.memset(cs[:], 1.0)
                    states[b] = (rowsums, cs, [])

                def stats(b, i):
                    rowsums, cs, e_tiles = states[b]
                    stats_tile(b, i, zts[b], rowsums, r_buf, cs, e_tiles)

                def cs_diag(b, t):
                    rowsums, cs, e_tiles = states[b]
                    colsum_tile(t, r_buf, cs, e_tiles)
                    if "nodiag" not in abl:
                        diag_patch(t, rowsums, e_tiles)

                prev = None
                for b in range(B_LOC):
                    if b == 0:
                        new_state(0)
                    for i in range(0 if b == 0 else PRE, NT):
                        stats(b, i)
                        if i >= CSLAG:
                            cs_diag(b, i - CSLAG)
                        if prev is not None and "statsonly" not in abl:
                            c_tile(prev[0], i - PRE, *prev[1:], False)
                    # crossing: b+1's first stats tiles between b's trailing
                    # colsums so ACT never starves on the cs->Ln chain
                    if b + 1 < B_LOC:
                        new_state(b + 1)
                        stats(b + 1, 0)
                        cs_diag(b, NT - CSLAG)
                        stats(b + 1, 1)
                        for t in range(NT - CSLAG + 1, NT):
                            cs_diag(b, t)
                    else:
                        for t in range(NT - CSLAG, NT):
                            cs_diag(b, t)
                    if prev is not None and "statsonly" not in abl:
                        for i in range(NT - PRE, NT):
                            c_tile(prev[0], i, *prev[1:], False)
                    rowsums, cs, e_tiles = states[b]
                    uv, vu, uv64, vu64 = batch_tail(b, cs)
                    prev = (b, uv, vu, uv64, vu64, e_tiles)
                if "statsonly" not in abl:
                    for i in range(NT):
                        c_tile(prev[0], i, *prev[1:], last_rep)

            if timing_trip is None:
                for rep in range(repeat):
                    emit_pipeline(rep == repeat - 1)
            else:
                with tc.For_i(0, timing_trip, 1):
                    emit_pipeline(False)
                tiny = singles.tile([2, 2], f32)
                nc.gpsimd.memset(tiny[:], 1.0)
                nc.sync.dma_start(tiny_out[:, :], tiny[:])

    nc.finalize()
    return nc


_NC_CACHE = None


def _get_nc() -> bass.Bass:
    global _NC_CACHE
    if _NC_CACHE is None:
        _NC_CACHE = build_nc()
    return _NC_CACHE


def _make_in_maps(xt: np.ndarray, W: np.ndarray, b: np.ndarray):
    xtT = np.ascontiguousarray(np.asarray(xt).transpose(0, 2, 1)).astype(np.float16)
    Wh = np.ascontiguousarray(np.asarray(W)).astype(np.float16)
    bh = np.ascontiguousarray(np.asarray(b)).reshape(H, 1).astype(np.float32)
    # cst cols: [Ln scale, Ln bias, uv-exp bias, vu-exp bias]
    # degree = 1.5 + 0.5*cs0 (plain-E colsum); cs1 = rowsum
    # u = exp(-0.5*ldeg - ln rs + ln 0.5), v = exp(-0.5*ldeg)
    ln_half = float(np.log(0.5))
    cst2 = np.array(
        [[0.5, 1.5, ln_half, 0.0], [1.0, 0.0, 0.0, ln_half]], dtype=np.float32
    )
    cuv2 = np.array(
        [[-0.5, -0.5, -0.5, -0.5], [-1.0, 0.0, 0.0, -1.0]], dtype=np.float32
    )
    # [34,4]: rows 32:34 replicate rows 0:2 (cs1bank column-half 1); pad rows
    # use scale 0 / bias 1 so the Ln of junk partitions stays finite
    cst = np.zeros((34, 4), dtype=np.float32)
    cst[:, 1] = 1.0
    cst[0:2] = cst2
    cst[32:34] = cst2
    cuv = np.zeros((34, 4), dtype=np.float32)
    cuv[0:2] = cuv2
    cuv[32:34] = cuv2
    return [
        {
            "xtT": xtT[B_LOC * k : B_LOC * (k + 1)],
            "W": Wh,
            "b": bh,
            "cst": cst,
            "cuv": cuv,
        }
        for k in range(N_CORES)
    ]


def run(xt, W, b, trace: bool = False):
    """Run on 8 NeuronCores; returns (out, BassKernelResults)."""
    res = run_bass_kernel_spmd(
        _get_nc(), _make_in_maps(xt, W, b), core_ids=list(range(N_CORES)), trace=trace
    )
    out = np.concatenate(
        [np.asarray(res.results[k]["out"]) for k in range(N_CORES)], axis=0
    )
    return out.astype(np.float32, copy=False), res


def kernel(xt: np.ndarray, W: np.ndarray, b: np.ndarray) -> np.ndarray:
    out, _ = run(xt, W, b, trace=False)
    return out



# revision 41
# speedup vs baseline: 1.0177x; 1.0177x over previous
"""AdaptiveAdjacency Bass kernel for 8 TRN2 NeuronCores.

Reference computation per batch b (N=1024 nodes, H=24 hidden):
    Z   = relu(xt @ W + b)                    (N, H)
    A   = Z @ Z.T                             (N, N)  -- symmetric!
    A   = 0.5*(softmax(A, -1) + softmax(A, -2)) + I
    deg = A.sum(-1);  out = A * deg^-1/2 [row] * deg^-1/2 [col]

Math used here (exploiting symmetry of A_raw):
    E = exp(A_raw - 40)            (shift is softmax-invariant; A_raw max ~54)
    softmax(A,-2) == softmax(A,-1).T, so with r = 1/rowsum(E):
        A_sym[n,m] = E[n,m] * 0.5*(r[n]+r[m]) + I
    Fold "+I" into E:  E' = E + diag(rowsum)  =>  out = E' * C with
        C[n,m] = u[n]v[m] + v[n]u[m],  u = 0.5*r*ds,  v = ds,
        ds = degree^-1/2, degree = 1 + 0.5*colsum(r[n]*E'[n,m])
    C is rank-2 -> one K=2 matmul per output tile; single elementwise
    multiply per output element (the only full-size DVE pass).

Sharding: data-parallel over B=32 across 8 cores (4 batches each);
W/b replicated. Host pre-transposes xt to (B, F, N) fp16 so the tiny
Linear runs as a natural PE matmul (contraction over F on partitions).
"""

import numpy as np

import concourse.bass as bass
import concourse.tile as tile
from concourse import bacc, mybir
from concourse.masks import make_identity
from concourse.bass_utils import run_bass_kernel_spmd

B_FULL = 32
B_LOC = 4  # batches per core
N = 1024
F = 64
H = 24
NT = N // 128  # 8 row tiles
CK = 512  # matmul free chunk (one PSUM bank)
NCK = N // CK
KSHIFT = -40.0  # softmax shift (global constant: softmax-invariant)
N_CORES = 8

f32 = mybir.dt.float32
bf16 = mybir.dt.bfloat16
fp16 = mybir.dt.float16
AF = mybir.ActivationFunctionType
ALU = mybir.AluOpType


_TABLES_PATCHED = False


def _force_single_act_table_set():
    """All activation funcs used here (Exp, Ln, Relu, Copy/Identity) live in
    the natural_log_exp_and_others set. bacc's table-load inserter picks the
    first set containing each function, which thrashes ~2.7us per switch
    between exp_and_others and natural_log. Strip those functions from every
    other set (indices must be preserved) so one table load covers the
    whole kernel."""
    global _TABLES_PATCHED
    if _TABLES_PATCHED:
        return
    _TABLES_PATCHED = True
    import concourse.hw_specs as hw_specs

    orig = hw_specs.get_activation_tables
    keep = {
        AF.Exp,
        AF.Ln,
        AF.Relu,
        AF.Copy,
        AF.Identity,
        AF.Square,
        AF.Abs,
        AF.Sign,
        AF.MemsetZero,
        AF.Is_finite,
    }
    target = "natural_log_exp_and_others"

    def patched(module_arch):
        tables = orig(module_arch)
        if target not in tables:
            return tables
        out = {}
        for name, funcs in tables.items():
            out[name] = funcs if name == target else (funcs - keep)
        return out

    hw_specs.get_activation_tables = patched
    bacc.get_activation_tables = patched


def build_nc(
    repeat: int = 1, timing_trip: int | None = None, ablate: str | None = None
) -> bass.Bass:
    """timing_trip=T builds a timing variant: the whole computation runs in
    an on-device For_i loop T times, writing to internal DRAM scratch with a
    tiny external output, so real device time per iteration can be measured
    by wall-clock differencing of two trip counts (fixed host/transfer costs
    cancel; code size is constant)."""
    abl = set(ablate.split(",")) if ablate else set()
    if "nocs1bank" not in abl:
        abl.add("cs1bank")  # single-bank colsum accumulator (frees a PSUM
        # bank so the C-matmul pool triple-buffers)
    _force_single_act_table_set()
    nc = bacc.Bacc()
    xtT = nc.declare_dram_parameter("xtT", [B_LOC, F, N], fp16, isOutput=False)
    Wd = nc.declare_dram_parameter("W", [F, H], fp16, isOutput=False)
    bd = nc.declare_dram_parameter("b", [H, 1], f32, isOutput=False)
    if timing_trip is None:
        outd = nc.declare_dram_parameter("out", [B_LOC, N, N], bf16, isOutput=True)
    else:
        outd = nc.dram_tensor("oscratch", [B_LOC, N, N], bf16)
        tiny_out = nc.declare_dram_parameter("out", [2, 2], f32, isOutput=True)
    # host constants (engine APs must start at partition 0, so these cannot
    # be built with sliced memsets):
    #   cst col 0/1: per-partition scale/bias for the fused colsum Ln
    #   cuv cols 0:2 = Cu, 2:4 = Cv (f32r lhsT for the log-mix matmuls)
    cstd = nc.declare_dram_parameter("cst", [34, 4], f32, isOutput=False)
    cuvd = nc.declare_dram_parameter("cuv", [34, 34], mybir.dt.float32r, isOutput=False)


    with tile.TileContext(nc) as tc:
        with (
            tc.tile_pool(name="singles", bufs=1) as singles,
            tc.tile_pool(name="zpool", bufs=B_LOC) as zpool,
            tc.tile_pool(name="epool", bufs=6 + 2 * NT) as epool,
            tc.tile_pool(name="vpool", bufs=2) as vpool,
            tc.tile_pool(name="opool", bufs=8 if (ablate and "op8" in ablate) else 6) as opool,
            tc.tile_pool(name="apool", bufs=2, space="PSUM") as apool,
            tc.tile_pool(name="cpool", bufs=2 if (ablate and "nocs1bank" in ablate) else 3, space="PSUM") as cpool,
            tc.tile_pool(name="cspool", bufs=1, space="PSUM") as cspool,
        ):
            wsb = singles.tile([F, H], fp16)
            nc.gpsimd.dma_start(wsb[:], Wd[:, :])
            bsb = singles.tile([H, 1], f32)
            nc.gpsimd.dma_start(bsb[:], bd[:, :])
            ident = singles.tile([128, 128], bf16)
            make_identity(nc, ident[:])
            cm40 = singles.tile([128, 1], f32)
            nc.gpsimd.memset(cm40[:], KSHIFT)
            # per-partition [scale, bias] for the fused colsum Ln:
            # row 0: ln(0.5*cs0 + 1.0)   row 1: ln(1.0*cs1 + 0.0)
            cstsb = singles.tile([34, 4], f32)
            nc.gpsimd.dma_start(cstsb[:], cstd[:, :])
            cuvsb = singles.tile([34, 34], mybir.dt.float32r)
            nc.gpsimd.dma_start(cuvsb[:], cuvd[:, :])

            # ---- Z^T = relu(W^T @ xt^T + b) : [H, N] fp16, all batches
            # upfront (fills otherwise-idle engines during pipeline fill and
            # removes the Z chain from the batch-boundary critical path) ----
            zts = []
            for b in range(B_LOC):
                xtsb = zpool.tile([F, N], fp16, tag="xt")
                nc.sync.dma_start(xtsb[:], xtT[b])
                zpsum = apool.tile([H, N], f32, tag="ps")
                for j in range(NCK):
                    nc.tensor.matmul(
                        zpsum[:, j * CK : (j + 1) * CK],
                        wsb[:],
                        xtsb[:, j * CK : (j + 1) * CK],
                        start=True,
                        stop=True,
                    )
                zt = zpool.tile([H, N], fp16, tag="zt")
                # relu on DVE (ACT is the bottleneck engine): (Zpre + b) max 0
                nc.vector.tensor_scalar(
                    zt[:], zpsum[:], bsb[:], 0.0, ALU.add, ALU.max
                )
                # replicas at partitions 32/64/96: A_raw matmuls spread over
                # all four PE row groups (K=24 fits a 32-row group) so up to
                # four streams run concurrently in the array
                zreps = [zt]
                for g in (32, 64, 96):
                    ztg = zpool.tile([g + H, N], fp16, tag=f"zt{g}")
                    nc.sync.dma_start(ztg[g : g + H, :], zt[:])
                    zreps.append(ztg)
                zts.append(zreps)
            # r_buf col 1 stays 1.0 forever; col 0 is overwritten by the
            # recips each batch. One tile + one memset for the whole kernel.
            r_buf = singles.tile([128, NT, 2], bf16)
            nc.gpsimd.memset(r_buf[:], 1.0)

            def stats_tile(b, i, ztpair, rowsums, r_buf, cs, e_tiles):
                """A_raw matmul, exp(+rowsum), r_i, colsum accumulate.

                The colsum runs on PLAIN E (not E' = E + diag(rowsum)):
                degree = 1.5 + 0.5*colsum(r*E) and cs1 = colsum(E) = rowsum
                (by symmetry), so the diag fix stays off this critical
                chain -- it is emitted later, anywhere before the c-phase.

                The (chunk, tile-parity) pair selects one of the four PE row
                groups via Z replicas at partitions 0/32/64/96, so the two
                chunk matmuls of a tile AND adjacent tiles all overlap in
                the systolic array."""
                zreps = ztpair
                apsum = apool.tile([128, N], f32, tag="ps")
                for j in range(NCK):
                    g = 2 * (i % 2) + j  # 0..3
                    z = zreps[g]
                    base = (32 * g, 32 * g + H)
                    zs = z[base[0] : base[1], :] if g else z[:, :]
                    nc.tensor.matmul(
                        apsum[:, j * CK : (j + 1) * CK],
                        zs[:, i * 128 : (i + 1) * 128],
                        zs[:, j * CK : (j + 1) * CK],
                        start=True,
                        stop=True,
                        tile_position=(32 * g, 0),
                    )
                et = epool.tile([128, N], bf16, tag="E")
                nc.scalar.activation(
                    et[:],
                    apsum[:],
                    AF.Exp,
                    bias=cm40[:],
                    accum_out=rowsums[:, i, :],
                )
                if i % 2 == 1:
                    # one paired reciprocal per two tiles: a [128,1] recip
                    # costs ~455ns of DVE but [128,2] only ~150 (fixed
                    # dispatch dominates); the pair forces CSLAG=2
                    with nc.allow_low_precision("bf16 r for colsum lhsT"):
                        nc.vector.reciprocal(
                            r_buf[:, i - 1 : i + 1, 0:1], rowsums[:, i - 1 : i + 1, :]
                        )
                e_tiles.append(et)

            def colsum_tile(i, r_buf, cs, e_tiles):
                """Accumulate cs += [r_i, 1]^T @ E_i. Issued CSLAG tiles
                behind the exp so the in-order PE never stalls on the
                exp/recip semaphores. cs1bank layout: column-half j lands at
                partition rows 32j:32j+2 (tile_position col group j), so the
                accumulator fits one PSUM bank and the two chunks stream in
                different array column groups."""
                if "nocolsum" in abl:
                    return
                lhs = ident[:, 0:2] if "csnor" in abl else r_buf[:, i, :]
                for j in range(NCK):
                    if "cs1bank" in abl:
                        out_ap = cs[32 * j : 32 * j + 2, :]
                        tp = (0, 32 * j)
                    else:
                        out_ap = cs[:, j * CK : (j + 1) * CK]
                        tp = None
                    nc.tensor.matmul(
                        out_ap,
                        lhs,
                        e_tiles[i][:, j * CK : (j + 1) * CK],
                        start=(i == 0),
                        stop=(i == NT - 1),
                        skip_group_check=True,
                        tile_position=tp,
                    )

            def batch_tail(b, cs):
                """degree -> ds; u, v vectors (free layout).
                degree = 1 + 0.5*cs0 ; v = ds = exp(-0.5*ln(degree))
                u = 0.5*r*ds = exp(-ln(cs1) - 0.5*ln(degree))  (cs1 = 2*rowsum)
                Engine ops must be lane-aligned (partition base 0), so the
                log-domain row mixing runs on the PE (K=2 f32r matmuls
                against a tiny constant lhsT), never across partitions."""
                uv = vpool.tile([2, N], bf16, tag="uv")  # [u; v] (lhsT source)
                vu = vpool.tile([2, N], bf16, tag="vu")  # [v; u] (rhs source)
                if "cs1bank" in abl:
                    lls = vpool.tile([34, CK], mybir.dt.float32r, tag="lls")
                    nc.scalar.activation(
                        lls[:], cs[:, :], AF.Ln, bias=cstsb[:, 1:2], scale=cstsb[:, 0:1]
                    )
                else:
                    lls = vpool.tile([2, N], mybir.dt.float32r, tag="lls")
                    nc.scalar.activation(
                        lls[:],
                        cs[:, :],
                        AF.Ln,
                        bias=cstsb[0:2, 1:2],
                        scale=cstsb[0:2, 0:1],
                    )
                # single M=34 log-mix matmul per column-half: the
                # zero-padded coefficient lhsT (cuv cols 0:2 -> uv rows at
                # psum 0:2, cols 32:34 -> vu rows at psum 32:34) lets ONE
                # [34,N] exp produce both tail vectors (ACT free-dim cost:
                # 1061ns total instead of 2x)
                lmix = apool.tile([34, N], f32, tag="ps")
                for j in range(NCK):
                    if "cs1bank" in abl:
                        lhs_ap = cuvsb[32 * j : 32 * j + 2, :]
                        rhs_ap = lls[32 * j : 32 * j + 2, :]
                        tp = (32 * j, 0)
                    else:
                        lhs_ap = cuvsb[0:2, :]
                        rhs_ap = lls[:, j * CK : (j + 1) * CK]
                        tp = None
                    nc.tensor.matmul(
                        lmix[:, j * CK : (j + 1) * CK],
                        lhs_ap,
                        rhs_ap,
                        start=True,
                        stop=True,
                        tile_position=tp,
                    )
                uvvu = vpool.tile([34, N], bf16, tag="uvvu")
                nc.scalar.activation(uvvu[:], lmix[:], AF.Exp, bias=cstsb[:, 2:3])
                uv = uvvu[0:2, :]  # [u; v] at base 0 (even-tile lhsT)
                # vu at base 0 (even-tile rhs) via DVE copy (658ns, no DMA
                # fixed cost); uv at base 32 (odd-tile lhsT) via scalar-HWDGE
                # DMA (first odd c-tile is one tile later: latency hidden).
                # Odd-tile rhs is uvvu[32:34] natively.
                vu = vpool.tile([2, N], bf16, tag="vu")
                nc.vector.tensor_copy(vu[:], uvvu[32:34, :])
                uv32 = vpool.tile([34, N], bf16, tag="uv32")
                nc.scalar.dma_start(uv32[32:34, :], uvvu[0:2, :])
                return uv, vu, uv32, uvvu[32:34, :]

            def diag_patch(i, rowsums, e_tiles):
                """E'[n,n] = E[n,n] + rowsum[n] (folds "+I" into the final
                multiply; one cheap TS+TT pair per tile, ~420ns of DVE).
                Measured faster on DVE than Pool with the triple-buffered
                cpool (Pool ops hold the shared DVE/Pool SBUF port).
                Ordered after the colsum's plain-E read by Tile's WAR
                tracking; needed before the c-phase."""
                et = e_tiles[i]
                eng = nc.gpsimd if "diagpool" in abl else nc.vector
                dtmp = vpool.tile([128, 128], bf16, tag="dtmp")
                eng.tensor_scalar_mul(dtmp[:], ident[:], rowsums[:, i, :])
                eng.tensor_tensor(
                    et[:, i * 128 : (i + 1) * 128],
                    et[:, i * 128 : (i + 1) * 128],
                    dtmp[:],
                    ALU.add,
                )

            def c_tile(b, i, uv, vu, uv64, vu64, e_tiles, last_batch):
                """C = u v^T + v u^T (K=2 matmul), out = E' * C, DMA out."""
                osb = opool.tile([128, N], bf16, tag="o")
                if i % 2 == 0 or "norep" in abl:
                    uvs, vus = uv, vu
                else:
                    uvs, vus = uv64[32:34, :], vu64
                for j in range(NCK):
                    cps = cpool.tile([128, CK], f32, tag="c")
                    nc.tensor.matmul(
                        cps[:],
                        uvs[:, i * 128 : (i + 1) * 128],
                        vus[:, j * CK : (j + 1) * CK],
                        start=True,
                        stop=True,
                    )
                    if last_batch and i < 5 and "nodrainsplit" not in abl:
                        # drain phase: ACT is idle -- route the PSUM exit
                        # through an ACT Copy so the multiply runs at the
                        # 2x both-SBUF TT rate (DVE 824ns/tile vs 1408)
                        csb = opool.tile([128, CK], bf16, tag="csb")
                        nc.scalar.activation(csb[:], cps[:], AF.Copy)
                        nc.vector.tensor_tensor(
                            osb[:, j * CK : (j + 1) * CK],
                            e_tiles[i][:, j * CK : (j + 1) * CK],
                            csb[:],
                            ALU.mult,
                        )
                    else:
                        nc.vector.tensor_tensor(
                            osb[:, j * CK : (j + 1) * CK],
                            e_tiles[i][:, j * CK : (j + 1) * CK],
                            cps[:],
                            ALU.mult,
                        )
                if "nodma" not in abl:
                    nc.sync.dma_start(outd[b, i * 128 : (i + 1) * 128, :], osb[:])


            # software pipeline: batch b's stats tiles interleave with batch
            # b-1's output tiles so PE/DVE/DMA trail ACT by one phase.
            # At each batch crossing, the next batch's first two stats tiles
            # are emitted BEFORE the trailing colsums + tail (keeps ACT fed
            # while the cs/Ln chain resolves), and the previous batch's last
            # two c-tiles land inside the crossing (keeps DVE fed during the
            # tail's Ln/lmix/exp latency).
            def emit_pipeline(last_rep):
                if "oldpipe" not in abl:
                    emit_pipeline_xing(last_rep)
                    return
                prev = None
                for b in range(B_LOC):
                    rowsums = vpool.tile([128, NT, 1], f32, tag="rowsums")
                    # cs[0,m] = sum_n r[n] E[n,m] = w; cs[1,m] = rowsum[m]
                    # (cs1bank: column-half j lives at partition rows 32j:32j+2
                    #  so the accumulator fits one PSUM bank)
                    if "cs1bank" in abl:
                        cs = cspool.tile([34, CK], f32, tag="cs")
                    else:
                        cs = cspool.tile([2, N], f32, tag="cs")
                    if "nocolsum" in abl:
                        nc.vector.memset(cs[:], 1.0)
                    e_tiles = []
                    if "csburst" in abl:
                        # colsums in two contiguous bursts: the K=128
                        # full-array matmuls drain the 32-row-group streams
                        # (A_raw/C) once per burst instead of per pair
                        for i in range(NT):
                            stats_tile(b, i, zts[b], rowsums, r_buf, cs, e_tiles)
                            if i == NT - 3:
                                for t in range(NT // 2):
                                    colsum_tile(t, r_buf, cs, e_tiles)
                                    if "nodiag" not in abl:
                                        diag_patch(t, rowsums, e_tiles)
                            if prev is not None and "statsonly" not in abl:
                                c_tile(prev[0], i, *prev[1:], False)
                        for t in range(NT // 2, NT):
                            colsum_tile(t, r_buf, cs, e_tiles)
                            if "nodiag" not in abl:
                                diag_patch(t, rowsums, e_tiles)
                    else:
                        CSLAG = 2 if "cslag2" in abl else (4 if "cslag4" in abl else 3)
                        for i in range(NT):
                            stats_tile(b, i, zts[b], rowsums, r_buf, cs, e_tiles)
                            if "csfirst" in abl:
                                if i >= CSLAG:
                                    colsum_tile(i - CSLAG, r_buf, cs, e_tiles)
                                    if "nodiag" not in abl:
                                        diag_patch(i - CSLAG, rowsums, e_tiles)
                                if prev is not None and "statsonly" not in abl:
                                    c_tile(prev[0], i, *prev[1:], False)
                            else:
                                if prev is not None and "statsonly" not in abl:
                                    c_tile(prev[0], i, *prev[1:], False)
                                if i >= CSLAG:
                                    colsum_tile(i - CSLAG, r_buf, cs, e_tiles)
                                    if "nodiag" not in abl:
                                        diag_patch(i - CSLAG, rowsums, e_tiles)
                        for i in range(NT - CSLAG, NT):
                            colsum_tile(i, r_buf, cs, e_tiles)
                            if "nodiag" not in abl:
                                diag_patch(i, rowsums, e_tiles)
                    uv, vu, uv64, vu64 = batch_tail(b, cs)
                    prev = (b, uv, vu, uv64, vu64, e_tiles)
                if "statsonly" not in abl:
                    for i in range(NT):
                        c_tile(prev[0], i, *prev[1:], last_rep)

            def emit_pipeline_xing(last_rep):
                PRE = 2  # next-batch stats tiles pre-emitted at the crossing
                CSLAG = 3
                states = {}

                def new_state(b):
                    rowsums = vpool.tile([128, NT, 1], f32, tag="rowsums")
                    if "cs1bank" in abl:
                        cs = cspool.tile([34, CK], f32, tag="cs")
                    else:
                        cs = cspool.tile([2, N], f32, tag="cs")
                    if "nocolsum" in abl:
                        nc.vector.memset(cs[:], 1.0)
                    states[b] = (rowsums, cs, [])

                def stats(b, i):
                    rowsums, cs, e_tiles = states[b]
                    stats_tile(b, i, zts[b], rowsums, r_buf, cs, e_tiles)

                def cs_diag(b, t):
                    rowsums, cs, e_tiles = states[b]
                    colsum_tile(t, r_buf, cs, e_tiles)
                    if "nodiag" not in abl:
                        diag_patch(t, rowsums, e_tiles)

                prev = None
                for b in range(B_LOC):
                    if b == 0:
                        new_state(0)
                    for i in range(0 if b == 0 else PRE, NT):
                        stats(b, i)
                        if i >= CSLAG:
                            cs_diag(b, i - CSLAG)
                        if prev is not None and "statsonly" not in abl:
                            c_tile(prev[0], i - PRE, *prev[1:], False)
                    # crossing: b+1's first stats tiles between b's trailing
                    # colsums so ACT never starves on the cs->Ln chain
                    if b + 1 < B_LOC:
                        new_state(b + 1)
                        stats(b + 1, 0)
                        cs_diag(b, NT - CSLAG)
                        stats(b + 1, 1)
                        for t in range(NT - CSLAG + 1, NT):
                            cs_diag(b, t)
                    else:
                        for t in range(NT - CSLAG, NT):
                            cs_diag(b, t)
                    if prev is not None and "statsonly" not in abl:
                        for i in range(NT - PRE, NT):
                            c_tile(prev[0], i, *prev[1:], False)
                    rowsums, cs, e_tiles = states[b]
                    uv, vu, uv64, vu64 = batch_tail(b, cs)
                    prev = (b, uv, vu, uv64, vu64, e_tiles)
                if "statsonly" not in abl:
                    for i in range(NT):
                        c_tile(prev[0], i, *prev[1:], last_rep)

            if timing_trip is None:
                for rep in range(repeat):
                    emit_pipeline(rep == repeat - 1)
            else:
                with tc.For_i(0, timing_trip, 1):
                    emit_pipeline(False)
                tiny = singles.tile([2, 2], f32)
                nc.gpsimd.memset(tiny[:], 1.0)
                nc.sync.dma_start(tiny_out[:, :], tiny[:])

    nc.finalize()
    return nc


_NC_CACHE = None


def _get_nc() -> bass.Bass:
    global _NC_CACHE
    if _NC_CACHE is None:
        _NC_CACHE = build_nc()
    return _NC_CACHE


def _make_in_maps(xt: np.ndarray, W: np.ndarray, b: np.ndarray):
    xtT = np.ascontiguousarray(np.asarray(xt).transpose(0, 2, 1)).astype(np.float16)
    Wh = np.ascontiguousarray(np.asarray(W)).astype(np.float16)
    bh = np.ascontiguousarray(np.asarray(b)).reshape(H, 1).astype(np.float32)
    # cst cols: [Ln scale, Ln bias, uv-exp bias, vu-exp bias]
    # degree = 1.5 + 0.5*cs0 (plain-E colsum); cs1 = rowsum
    # u = exp(-0.5*ldeg - ln rs + ln 0.5), v = exp(-0.5*ldeg)
    ln_half = float(np.log(0.5))
    cst2 = np.array(
        [[0.5, 1.5, ln_half, 0.0], [1.0, 0.0, 0.0, ln_half]], dtype=np.float32
    )
    cuv2 = np.array(
        [[-0.5, -0.5, -0.5, -0.5], [-1.0, 0.0, 0.0, -1.0]], dtype=np.float32
    )
    # [34,4]: rows 32:34 replicate rows 0:2 (cs1bank column-half 1); pad rows
    # use scale 0 / bias 1 so the Ln of junk partitions stays finite
    cst = np.zeros((34, 4), dtype=np.float32)
    cst[:, 1] = 1.0
    cst[0:2] = cst2
    cst[32:34] = cst2
    # combined-exp bias col 2: rows 0:2 = uv bias [ln1/2, 0], rows 32:34 =
    # vu bias [0, ln1/2]; pad rows 0 (exp(0)=1, finite, never read)
    cst[:, 2] = 0.0
    cst[0, 2] = ln_half
    cst[33, 2] = ln_half
    # lmix coefficients: out row 0 = u-pre (-0.5*ldeg - lnrs), row 1 =
    # v-pre (-0.5*ldeg); rows 32/33 = v-pre/u-pre ([v; u] order)
    cuvL = np.zeros((2, 34), dtype=np.float32)
    cuvL[:, 0] = (-0.5, -1.0)
    cuvL[:, 1] = (-0.5, 0.0)
    cuvL[:, 32] = (-0.5, 0.0)
    cuvL[:, 33] = (-0.5, -1.0)
    cuv = np.zeros((34, 34), dtype=np.float32)
    cuv[0:2] = cuvL
    cuv[32:34] = cuvL
    return [
        {
            "xtT": xtT[B_LOC * k : B_LOC * (k + 1)],
            "W": Wh,
            "b": bh,
            "cst": cst,
            "cuv": cuv,
        }
        for k in range(N_CORES)
    ]


def run(xt, W, b, trace: bool = False):
    """Run on 8 NeuronCores; returns (out, BassKernelResults)."""
    res = run_bass_kernel_spmd(
        _get_nc(), _make_in_maps(xt, W, b), core_ids=list(range(N_CORES)), trace=trace
    )
    out = np.concatenate(
        [np.asarray(res.results[k]["out"]) for k in range(N_CORES)], axis=0
    )
    return out.astype(np.float32, copy=False), res


def kernel(xt: np.ndarray, W: np.ndarray, b: np.ndarray) -> np.ndarray:
    out, _ = run(xt, W, b, trace=False)
    return out



# revision 42
# speedup vs baseline: 1.0461x; 1.0279x over previous
"""AdaptiveAdjacency Bass kernel for 8 TRN2 NeuronCores.

Reference computation per batch b (N=1024 nodes, H=24 hidden):
    Z   = relu(xt @ W + b)                    (N, H)
    A   = Z @ Z.T                             (N, N)  -- symmetric!
    A   = 0.5*(softmax(A, -1) + softmax(A, -2)) + I
    deg = A.sum(-1);  out = A * deg^-1/2 [row] * deg^-1/2 [col]

Math used here (exploiting symmetry of A_raw):
    E = exp(A_raw - 40)            (shift is softmax-invariant; A_raw max ~54)
    softmax(A,-2) == softmax(A,-1).T, so with r = 1/rowsum(E):
        A_sym[n,m] = E[n,m] * 0.5*(r[n]+r[m]) + I
    Fold "+I" into E:  E' = E + diag(rowsum)  =>  out = E' * C with
        C[n,m] = u[n]v[m] + v[n]u[m],  u = 0.5*r*ds,  v = ds,
        ds = degree^-1/2, degree = 1 + 0.5*colsum(r[n]*E'[n,m])
    C is rank-2 -> one K=2 matmul per output tile; single elementwise
    multiply per output element (the only full-size DVE pass).

Sharding: data-parallel over B=32 across 8 cores (4 batches each);
W/b replicated. Host pre-transposes xt to (B, F, N) fp16 so the tiny
Linear runs as a natural PE matmul (contraction over F on partitions).
"""

import numpy as np

import concourse.bass as bass
import concourse.tile as tile
from concourse import bacc, mybir
from concourse.masks import make_identity
from concourse.bass_utils import run_bass_kernel_spmd

B_FULL = 32
B_LOC = 4  # batches per core
N = 1024
F = 64
H = 24
NT = N // 128  # 8 row tiles
CK = 512  # matmul free chunk (one PSUM bank)
NCK = N // CK
KSHIFT = -40.0  # softmax shift (global constant: softmax-invariant)
N_CORES = 8

f32 = mybir.dt.float32
bf16 = mybir.dt.bfloat16
fp16 = mybir.dt.float16
AF = mybir.ActivationFunctionType
ALU = mybir.AluOpType


_TABLES_PATCHED = False


def _force_single_act_table_set():
    """All activation funcs used here (Exp, Ln, Relu, Copy/Identity) live in
    the natural_log_exp_and_others set. bacc's table-load inserter picks the
    first set containing each function, which thrashes ~2.7us per switch
    between exp_and_others and natural_log. Strip those functions from every
    other set (indices must be preserved) so one table load covers the
    whole kernel."""
    global _TABLES_PATCHED
    if _TABLES_PATCHED:
        return
    _TABLES_PATCHED = True
    import concourse.hw_specs as hw_specs

    orig = hw_specs.get_activation_tables
    keep = {
        AF.Exp,
        AF.Ln,
        AF.Relu,
        AF.Copy,
        AF.Identity,
        AF.Square,
        AF.Abs,
        AF.Sign,
        AF.MemsetZero,
        AF.Is_finite,
    }
    target = "natural_log_exp_and_others"

    def patched(module_arch):
        tables = orig(module_arch)
        if target not in tables:
            return tables
        out = {}
        for name, funcs in tables.items():
            out[name] = funcs if name == target else (funcs - keep)
        return out

    hw_specs.get_activation_tables = patched
    bacc.get_activation_tables = patched


def build_nc(
    repeat: int = 1, timing_trip: int | None = None, ablate: str | None = None
) -> bass.Bass:
    """timing_trip=T builds a timing variant: the whole computation runs in
    an on-device For_i loop T times, writing to internal DRAM scratch with a
    tiny external output, so real device time per iteration can be measured
    by wall-clock differencing of two trip counts (fixed host/transfer costs
    cancel; code size is constant)."""
    abl = set(ablate.split(",")) if ablate else set()
    if "nocs1bank" not in abl:
        abl.add("cs1bank")  # single-bank colsum accumulator (frees a PSUM
        # bank so the C-matmul pool triple-buffers)
    _force_single_act_table_set()
    nc = bacc.Bacc()
    xtT = nc.declare_dram_parameter("xtT", [B_LOC, F, N], fp16, isOutput=False)
    Wd = nc.declare_dram_parameter("W", [F, H], fp16, isOutput=False)
    bd = nc.declare_dram_parameter("b", [H, 1], f32, isOutput=False)
    if timing_trip is None:
        outd = nc.declare_dram_parameter("out", [B_LOC, N, N], bf16, isOutput=True)
    else:
        outd = nc.dram_tensor("oscratch", [B_LOC, N, N], bf16)
        tiny_out = nc.declare_dram_parameter("out", [2, 2], f32, isOutput=True)
    # host constants (engine APs must start at partition 0, so these cannot
    # be built with sliced memsets):
    #   cst col 0/1: per-partition scale/bias for the fused colsum Ln
    #   cuv cols 0:2 = Cu, 2:4 = Cv (f32r lhsT for the log-mix matmuls)
    cstd = nc.declare_dram_parameter("cst", [34, 4], f32, isOutput=False)
    cuvd = nc.declare_dram_parameter("cuv", [34, 34], mybir.dt.float32r, isOutput=False)


    with tile.TileContext(nc) as tc:
        with (
            tc.tile_pool(name="singles", bufs=1) as singles,
            tc.tile_pool(name="zpool", bufs=B_LOC) as zpool,
            tc.tile_pool(name="epool", bufs=6 + 2 * NT) as epool,
            tc.tile_pool(name="vpool", bufs=2) as vpool,
            tc.tile_pool(name="opool", bufs=8 if (ablate and "op8" in ablate) else 6) as opool,
            tc.tile_pool(name="apool", bufs=2, space="PSUM") as apool,
            tc.tile_pool(name="cpool", bufs=2 if (ablate and "nocs1bank" in ablate) else 3, space="PSUM") as cpool,
            tc.tile_pool(name="cspool", bufs=1, space="PSUM") as cspool,
        ):
            wsb = singles.tile([F, H], fp16)
            nc.gpsimd.dma_start(wsb[:], Wd[:, :])
            bsb = singles.tile([H, 1], f32)
            nc.gpsimd.dma_start(bsb[:], bd[:, :])
            ident = singles.tile([128, 128], bf16)
            make_identity(nc, ident[:])
            cm40 = singles.tile([128, 1], f32)
            nc.gpsimd.memset(cm40[:], KSHIFT)
            # per-partition [scale, bias] for the fused colsum Ln:
            # row 0: ln(0.5*cs0 + 1.0)   row 1: ln(1.0*cs1 + 0.0)
            cstsb = singles.tile([34, 4], f32)
            nc.gpsimd.dma_start(cstsb[:], cstd[:, :])
            cuvsb = singles.tile([34, 34], mybir.dt.float32r)
            nc.gpsimd.dma_start(cuvsb[:], cuvd[:, :])

            # ---- Z^T = relu(W^T @ xt^T + b) : [H, N] fp16, all batches
            # upfront (fills otherwise-idle engines during pipeline fill and
            # removes the Z chain from the batch-boundary critical path) ----
            zts = []
            for b in range(B_LOC):
                xtsb = zpool.tile([F, N], fp16, tag="xt")
                nc.sync.dma_start(xtsb[:], xtT[b])
                zpsum = apool.tile([H, N], f32, tag="ps")
                for j in range(NCK):
                    nc.tensor.matmul(
                        zpsum[:, j * CK : (j + 1) * CK],
                        wsb[:],
                        xtsb[:, j * CK : (j + 1) * CK],
                        start=True,
                        stop=True,
                    )
                zt = zpool.tile([H, N], fp16, tag="zt")
                # relu on DVE (ACT is the bottleneck engine): (Zpre + b) max 0
                nc.vector.tensor_scalar(
                    zt[:], zpsum[:], bsb[:], 0.0, ALU.add, ALU.max
                )
                # replicas at partitions 32/64/96: A_raw matmuls spread over
                # all four PE row groups (K=24 fits a 32-row group) so up to
                # four streams run concurrently in the array
                zreps = [zt]
                for g in (32, 64, 96):
                    ztg = zpool.tile([g + H, N], fp16, tag=f"zt{g}")
                    nc.sync.dma_start(ztg[g : g + H, :], zt[:])
                    zreps.append(ztg)
                zts.append(zreps)
            # r_buf col 1 stays 1.0 forever; col 0 is overwritten by the
            # recips each batch. One tile + one memset for the whole kernel.
            r_buf = singles.tile([128, NT, 2], bf16)
            nc.gpsimd.memset(r_buf[:], 1.0)

            def stats_tile(b, i, ztpair, rowsums, r_buf, cs, e_tiles):
                """A_raw matmul, exp(+rowsum), r_i, colsum accumulate.

                The colsum runs on PLAIN E (not E' = E + diag(rowsum)):
                degree = 1.5 + 0.5*colsum(r*E) and cs1 = colsum(E) = rowsum
                (by symmetry), so the diag fix stays off this critical
                chain -- it is emitted later, anywhere before the c-phase.

                The (chunk, tile-parity) pair selects one of the four PE row
                groups via Z replicas at partitions 0/32/64/96, so the two
                chunk matmuls of a tile AND adjacent tiles all overlap in
                the systolic array."""
                zreps = ztpair
                apsum = apool.tile([128, N], f32, tag="ps")
                for j in range(NCK):
                    g = 2 * (i % 2) + j  # 0..3
                    z = zreps[g]
                    base = (32 * g, 32 * g + H)
                    zs = z[base[0] : base[1], :] if g else z[:, :]
                    nc.tensor.matmul(
                        apsum[:, j * CK : (j + 1) * CK],
                        zs[:, i * 128 : (i + 1) * 128],
                        zs[:, j * CK : (j + 1) * CK],
                        start=True,
                        stop=True,
                        tile_position=(32 * g, 0),
                    )
                et = epool.tile([128, N], bf16, tag="E")
                nc.scalar.activation(
                    et[:],
                    apsum[:],
                    AF.Exp,
                    bias=cm40[:],
                    accum_out=rowsums[:, i, :],
                )
                if i % 2 == 1:
                    # one paired reciprocal per two tiles: a [128,1] recip
                    # costs ~455ns of DVE but [128,2] only ~150 (fixed
                    # dispatch dominates); the pair forces CSLAG=2
                    with nc.allow_low_precision("bf16 r for colsum lhsT"):
                        nc.vector.reciprocal(
                            r_buf[:, i - 1 : i + 1, 0:1], rowsums[:, i - 1 : i + 1, :]
                        )
                e_tiles.append(et)

            def colsum_tile(i, r_buf, cs, e_tiles):
                """Accumulate cs += [r_i, 1]^T @ E_i. Issued CSLAG tiles
                behind the exp so the in-order PE never stalls on the
                exp/recip semaphores. cs1bank layout: column-half j lands at
                partition rows 32j:32j+2 (tile_position col group j), so the
                accumulator fits one PSUM bank and the two chunks stream in
                different array column groups."""
                if "nocolsum" in abl:
                    return
                lhs = ident[:, 0:2] if "csnor" in abl else r_buf[:, i, :]
                for j in range(NCK):
                    if "cs1bank" in abl:
                        out_ap = cs[32 * j : 32 * j + 2, :]
                        tp = (0, 32 * j)
                    else:
                        out_ap = cs[:, j * CK : (j + 1) * CK]
                        tp = None
                    nc.tensor.matmul(
                        out_ap,
                        lhs,
                        e_tiles[i][:, j * CK : (j + 1) * CK],
                        start=(i == 0),
                        stop=(i == NT - 1),
                        skip_group_check=True,
                        tile_position=tp,
                    )

            def batch_tail(b, cs):
                """degree -> ds; u, v vectors (free layout).
                degree = 1 + 0.5*cs0 ; v = ds = exp(-0.5*ln(degree))
                u = 0.5*r*ds = exp(-ln(cs1) - 0.5*ln(degree))  (cs1 = 2*rowsum)
                Engine ops must be lane-aligned (partition base 0), so the
                log-domain row mixing runs on the PE (K=2 f32r matmuls
                against a tiny constant lhsT), never across partitions."""
                uv = vpool.tile([2, N], bf16, tag="uv")  # [u; v] (lhsT source)
                vu = vpool.tile([2, N], bf16, tag="vu")  # [v; u] (rhs source)
                if "cs1bank" in abl:
                    lls = vpool.tile([34, CK], mybir.dt.float32r, tag="lls")
                    nc.scalar.activation(
                        lls[:], cs[:, :], AF.Ln, bias=cstsb[:, 1:2], scale=cstsb[:, 0:1]
                    )
                else:
                    lls = vpool.tile([2, N], mybir.dt.float32r, tag="lls")
                    nc.scalar.activation(
                        lls[:],
                        cs[:, :],
                        AF.Ln,
                        bias=cstsb[0:2, 1:2],
                        scale=cstsb[0:2, 0:1],
                    )
                # single M=34 log-mix matmul per column-half: the
                # zero-padded coefficient lhsT (cuv cols 0:2 -> uv rows at
                # psum 0:2, cols 32:34 -> vu rows at psum 32:34) lets ONE
                # [34,N] exp produce both tail vectors (ACT free-dim cost:
                # 1061ns total instead of 2x)
                lmix = apool.tile([34, N], f32, tag="ps")
                for j in range(NCK):
                    if "cs1bank" in abl:
                        lhs_ap = cuvsb[32 * j : 32 * j + 2, :]
                        rhs_ap = lls[32 * j : 32 * j + 2, :]
                        tp = (32 * j, 0)
                    else:
                        lhs_ap = cuvsb[0:2, :]
                        rhs_ap = lls[:, j * CK : (j + 1) * CK]
                        tp = None
                    nc.tensor.matmul(
                        lmix[:, j * CK : (j + 1) * CK],
                        lhs_ap,
                        rhs_ap,
                        start=True,
                        stop=True,
                        tile_position=tp,
                    )
                uvvu = vpool.tile([34, N], bf16, tag="uvvu")
                nc.scalar.activation(uvvu[:], lmix[:], AF.Exp, bias=cstsb[:, 2:3])
                uv = uvvu[0:2, :]  # [u; v] at base 0 (even-tile lhsT)
                # vu at base 0 (even-tile rhs) via DVE copy (658ns, no DMA
                # fixed cost); uv at base 32 (odd-tile lhsT) via scalar-HWDGE
                # DMA (first odd c-tile is one tile later: latency hidden).
                # Odd-tile rhs is uvvu[32:34] natively.
                vu = vpool.tile([2, N], bf16, tag="vu")
                nc.vector.tensor_copy(vu[:], uvvu[32:34, :])
                uv32 = vpool.tile([34, N], bf16, tag="uv32")
                nc.scalar.dma_start(uv32[32:34, :], uvvu[0:2, :])
                return uv, vu, uv32, uvvu[32:34, :]

            def diag_patch(i, rowsums, e_tiles):
                """E'[n,n] = E[n,n] + rowsum[n] (folds "+I" into the final
                multiply; one cheap TS+TT pair per tile, ~420ns of DVE).
                Measured faster on DVE than Pool with the triple-buffered
                cpool (Pool ops hold the shared DVE/Pool SBUF port).
                Ordered after the colsum's plain-E read by Tile's WAR
                tracking; needed before the c-phase."""
                et = e_tiles[i]
                eng = nc.gpsimd if "diagpool" in abl else nc.vector
                ts_eng = nc.gpsimd if ("diagts_pool" in abl or "diagpool" in abl) else nc.vector
                dtmp = vpool.tile([128, 128], bf16, tag="dtmp")
                ts_eng.tensor_scalar_mul(dtmp[:], ident[:], rowsums[:, i, :])
                eng.tensor_tensor(
                    et[:, i * 128 : (i + 1) * 128],
                    et[:, i * 128 : (i + 1) * 128],
                    dtmp[:],
                    ALU.add,
                )

            def c_tile(b, i, uv, vu, uv64, vu64, e_tiles, last_batch):
                """C = u v^T + v u^T (K=2 matmul), out = E' * C, DMA out."""
                osb = opool.tile([128, N], bf16, tag="o")
                if i % 2 == 0 or "norep" in abl:
                    uvs, vus = uv, vu
                else:
                    uvs, vus = uv64[32:34, :], vu64
                for j in range(NCK):
                    cps = cpool.tile([128, CK], f32, tag="c")
                    nc.tensor.matmul(
                        cps[:],
                        uvs[:, i * 128 : (i + 1) * 128],
                        vus[:, j * CK : (j + 1) * CK],
                        start=True,
                        stop=True,
                    )
                    if last_batch and i < (6 if "drain6" in abl else 5) and "nodrainsplit" not in abl:
                        # drain phase: ACT is idle -- route the PSUM exit
                        # through an ACT Copy so the multiply runs at the
                        # 2x both-SBUF TT rate (DVE 824ns/tile vs 1408)
                        csb = opool.tile([128, CK], bf16, tag="csb")
                        nc.scalar.activation(csb[:], cps[:], AF.Copy)
                        nc.vector.tensor_tensor(
                            osb[:, j * CK : (j + 1) * CK],
                            e_tiles[i][:, j * CK : (j + 1) * CK],
                            csb[:],
                            ALU.mult,
                        )
                    else:
                        nc.vector.tensor_tensor(
                            osb[:, j * CK : (j + 1) * CK],
                            e_tiles[i][:, j * CK : (j + 1) * CK],
                            cps[:],
                            ALU.mult,
                        )
                if "nodma" not in abl:
                    nc.sync.dma_start(outd[b, i * 128 : (i + 1) * 128, :], osb[:])


            # software pipeline: batch b's stats tiles interleave with batch
            # b-1's output tiles so PE/DVE/DMA trail ACT by one phase.
            # At each batch crossing, the next batch's first two stats tiles
            # are emitted BEFORE the trailing colsums + tail (keeps ACT fed
            # while the cs/Ln chain resolves), and the previous batch's last
            # two c-tiles land inside the crossing (keeps DVE fed during the
            # tail's Ln/lmix/exp latency).
            def emit_pipeline(last_rep):
                if "oldpipe" not in abl:
                    emit_pipeline_xing(last_rep)
                    return
                prev = None
                for b in range(B_LOC):
                    rowsums = vpool.tile([128, NT, 1], f32, tag="rowsums")
                    # cs[0,m] = sum_n r[n] E[n,m] = w; cs[1,m] = rowsum[m]
                    # (cs1bank: column-half j lives at partition rows 32j:32j+2
                    #  so the accumulator fits one PSUM bank)
                    if "cs1bank" in abl:
                        cs = cspool.tile([34, CK], f32, tag="cs")
                    else:
                        cs = cspool.tile([2, N], f32, tag="cs")
                    if "nocolsum" in abl:
                        nc.vector.memset(cs[:], 1.0)
                    e_tiles = []
                    if "csburst" in abl:
                        # colsums in two contiguous bursts: the K=128
                        # full-array matmuls drain the 32-row-group streams
                        # (A_raw/C) once per burst instead of per pair
                        for i in range(NT):
                            stats_tile(b, i, zts[b], rowsums, r_buf, cs, e_tiles)
                            if i == NT - 3:
                                for t in range(NT // 2):
                                    colsum_tile(t, r_buf, cs, e_tiles)
                                    if "nodiag" not in abl:
                                        diag_patch(t, rowsums, e_tiles)
                            if prev is not None and "statsonly" not in abl:
                                c_tile(prev[0], i, *prev[1:], False)
                        for t in range(NT // 2, NT):
                            colsum_tile(t, r_buf, cs, e_tiles)
                            if "nodiag" not in abl:
                                diag_patch(t, rowsums, e_tiles)
                    else:
                        CSLAG = 2 if "cslag2" in abl else (4 if "cslag4" in abl else 3)
                        for i in range(NT):
                            stats_tile(b, i, zts[b], rowsums, r_buf, cs, e_tiles)
                            if "csfirst" in abl:
                                if i >= CSLAG:
                                    colsum_tile(i - CSLAG, r_buf, cs, e_tiles)
                                    if "nodiag" not in abl:
                                        diag_patch(i - CSLAG, rowsums, e_tiles)
                                if prev is not None and "statsonly" not in abl:
                                    c_tile(prev[0], i, *prev[1:], False)
                            else:
                                if prev is not None and "statsonly" not in abl:
                                    c_tile(prev[0], i, *prev[1:], False)
                                if i >= CSLAG:
                                    colsum_tile(i - CSLAG, r_buf, cs, e_tiles)
                                    if "nodiag" not in abl:
                                        diag_patch(i - CSLAG, rowsums, e_tiles)
                        for i in range(NT - CSLAG, NT):
                            colsum_tile(i, r_buf, cs, e_tiles)
                            if "nodiag" not in abl:
                                diag_patch(i, rowsums, e_tiles)
                    uv, vu, uv64, vu64 = batch_tail(b, cs)
                    prev = (b, uv, vu, uv64, vu64, e_tiles)
                if "statsonly" not in abl:
                    for i in range(NT):
                        c_tile(prev[0], i, *prev[1:], last_rep)

            def emit_pipeline_xing(last_rep):
                PRE = 2  # next-batch stats tiles pre-emitted at the crossing
                CSLAG = 3
                states = {}

                def new_state(b):
                    rowsums = vpool.tile([128, NT, 1], f32, tag="rowsums")
                    if "cs1bank" in abl:
                        cs = cspool.tile([34, CK], f32, tag="cs")
                    else:
                        cs = cspool.tile([2, N], f32, tag="cs")
                    if "nocolsum" in abl:
                        nc.vector.memset(cs[:], 1.0)
                    states[b] = (rowsums, cs, [])

                def stats(b, i):
                    rowsums, cs, e_tiles = states[b]
                    stats_tile(b, i, zts[b], rowsums, r_buf, cs, e_tiles)

                def cs_diag(b, t):
                    rowsums, cs, e_tiles = states[b]
                    colsum_tile(t, r_buf, cs, e_tiles)
                    if "nodiag" not in abl:
                        diag_patch(t, rowsums, e_tiles)

                prev = None
                for b in range(B_LOC):
                    if b == 0:
                        new_state(0)
                    for i in range(0 if b == 0 else PRE, NT):
                        stats(b, i)
                        if i >= CSLAG:
                            cs_diag(b, i - CSLAG)
                        if prev is not None and "statsonly" not in abl:
                            c_tile(prev[0], i - PRE, *prev[1:], False)
                    # crossing: b+1's first stats tiles between b's trailing
                    # colsums so ACT never starves on the cs->Ln chain
                    if b + 1 < B_LOC:
                        new_state(b + 1)
                        stats(b + 1, 0)
                        cs_diag(b, NT - CSLAG)
                        stats(b + 1, 1)
                        for t in range(NT - CSLAG + 1, NT):
                            cs_diag(b, t)
                    else:
                        for t in range(NT - CSLAG, NT):
                            cs_diag(b, t)
                    if prev is not None and "statsonly" not in abl:
                        for i in range(NT - PRE, NT):
                            c_tile(prev[0], i, *prev[1:], False)
                    rowsums, cs, e_tiles = states[b]
                    uv, vu, uv64, vu64 = batch_tail(b, cs)
                    prev = (b, uv, vu, uv64, vu64, e_tiles)
                if "statsonly" not in abl:
                    for i in range(NT):
                        c_tile(prev[0], i, *prev[1:], last_rep)

            if timing_trip is None:
                for rep in range(repeat):
                    emit_pipeline(rep == repeat - 1)
            else:
                with tc.For_i(0, timing_trip, 1):
                    emit_pipeline(False)
                tiny = singles.tile([2, 2], f32)
                nc.gpsimd.memset(tiny[:], 1.0)
                nc.sync.dma_start(tiny_out[:, :], tiny[:])

    nc.finalize()
    return nc


_NC_CACHE = None


def _get_nc() -> bass.Bass:
    global _NC_CACHE
    if _NC_CACHE is None:
        _NC_CACHE = build_nc()
    return _NC_CACHE


def _make_in_maps(xt: np.ndarray, W: np.ndarray, b: np.ndarray):
    xtT = np.ascontiguousarray(np.asarray(xt).transpose(0, 2, 1)).astype(np.float16)
    Wh = np.ascontiguousarray(np.asarray(W)).astype(np.float16)
    bh = np.ascontiguousarray(np.asarray(b)).reshape(H, 1).astype(np.float32)
    # cst cols: [Ln scale, Ln bias, uv-exp bias, vu-exp bias]
    # degree = 1.5 + 0.5*cs0 (plain-E colsum); cs1 = rowsum
    # u = exp(-0.5*ldeg - ln rs + ln 0.5), v = exp(-0.5*ldeg)
    ln_half = float(np.log(0.5))
    cst2 = np.array(
        [[0.5, 1.5, ln_half, 0.0], [1.0, 0.0, 0.0, ln_half]], dtype=np.float32
    )
    cuv2 = np.array(
        [[-0.5, -0.5, -0.5, -0.5], [-1.0, 0.0, 0.0, -1.0]], dtype=np.float32
    )
    # [34,4]: rows 32:34 replicate rows 0:2 (cs1bank column-half 1); pad rows
    # use scale 0 / bias 1 so the Ln of junk partitions stays finite
    cst = np.zeros((34, 4), dtype=np.float32)
    cst[:, 1] = 1.0
    cst[0:2] = cst2
    cst[32:34] = cst2
    # combined-exp bias col 2: rows 0:2 = uv bias [ln1/2, 0], rows 32:34 =
    # vu bias [0, ln1/2]; pad rows 0 (exp(0)=1, finite, never read)
    cst[:, 2] = 0.0
    cst[0, 2] = ln_half
    cst[33, 2] = ln_half
    # lmix coefficients: out row 0 = u-pre (-0.5*ldeg - lnrs), row 1 =
    # v-pre (-0.5*ldeg); rows 32/33 = v-pre/u-pre ([v; u] order)
    cuvL = np.zeros((2, 34), dtype=np.float32)
    cuvL[:, 0] = (-0.5, -1.0)
    cuvL[:, 1] = (-0.5, 0.0)
    cuvL[:, 32] = (-0.5, 0.0)
    cuvL[:, 33] = (-0.5, -1.0)
    cuv = np.zeros((34, 34), dtype=np.float32)
    cuv[0:2] = cuvL
    cuv[32:34] = cuvL
    return [
        {
            "xtT": xtT[B_LOC * k : B_LOC * (k + 1)],
            "W": Wh,
            "b": bh,
            "cst": cst,
            "cuv": cuv,
        }
        for k in range(N_CORES)
    ]


def run(xt, W, b, trace: bool = False):
    """Run on 8 NeuronCores; returns (out, BassKernelResults)."""
    res = run_bass_kernel_spmd(
        _get_nc(), _make_in_maps(xt, W, b), core_ids=list(range(N_CORES)), trace=trace
    )
    out = np.concatenate(
        [np.asarray(res.results[k]["out"]) for k in range(N_CORES)], axis=0
    )
    return out.astype(np.float32, copy=False), res


def kernel(xt: np.ndarray, W: np.ndarray, b: np.ndarray) -> np.ndarray:
    out, _ = run(xt, W, b, trace=False)
    return out



# revision 43
# speedup vs baseline: 1.1109x; 1.0619x over previous
"""AdaptiveAdjacency Bass kernel for 8 TRN2 NeuronCores.

Reference computation per batch b (N=1024 nodes, H=24 hidden):
    Z   = relu(xt @ W + b)                    (N, H)
    A   = Z @ Z.T                             (N, N)  -- symmetric!
    A   = 0.5*(softmax(A, -1) + softmax(A, -2)) + I
    deg = A.sum(-1);  out = A * deg^-1/2 [row] * deg^-1/2 [col]

Math used here (exploiting symmetry of A_raw):
    E = exp(A_raw - 40)            (shift is softmax-invariant; A_raw max ~54)
    softmax(A,-2) == softmax(A,-1).T, so with r = 1/rowsum(E):
        A_sym[n,m] = E[n,m] * 0.5*(r[n]+r[m]) + I
    Fold "+I" into E:  E' = E + diag(rowsum)  =>  out = E' * C with
        C[n,m] = u[n]v[m] + v[n]u[m],  u = 0.5*r*ds,  v = ds,
        ds = degree^-1/2, degree = 1 + 0.5*colsum(r[n]*E'[n,m])
    C is rank-2 -> one K=2 matmul per output tile; single elementwise
    multiply per output element (the only full-size DVE pass).

Sharding: data-parallel over B=32 across 8 cores (4 batches each);
W/b replicated. Host pre-transposes xt to (B, F, N) fp16 so the tiny
Linear runs as a natural PE matmul (contraction over F on partitions).
"""

import numpy as np

import concourse.bass as bass
import concourse.tile as tile
from concourse import bacc, mybir
from concourse.masks import make_identity
from concourse.bass_utils import run_bass_kernel_spmd

B_FULL = 32
B_LOC = 4  # batches per core
N = 1024
F = 64
H = 24
NT = N // 128  # 8 row tiles
CK = 512  # matmul free chunk (one PSUM bank)
NCK = N // CK
KSHIFT = -40.0  # softmax shift (global constant: softmax-invariant)
N_CORES = 8

f32 = mybir.dt.float32
bf16 = mybir.dt.bfloat16
fp16 = mybir.dt.float16
AF = mybir.ActivationFunctionType
ALU = mybir.AluOpType


_TABLES_PATCHED = False


def _force_single_act_table_set():
    """All activation funcs used here (Exp, Ln, Relu, Copy/Identity) live in
    the natural_log_exp_and_others set. bacc's table-load inserter picks the
    first set containing each function, which thrashes ~2.7us per switch
    between exp_and_others and natural_log. Strip those functions from every
    other set (indices must be preserved) so one table load covers the
    whole kernel."""
    global _TABLES_PATCHED
    if _TABLES_PATCHED:
        return
    _TABLES_PATCHED = True
    import concourse.hw_specs as hw_specs

    orig = hw_specs.get_activation_tables
    keep = {
        AF.Exp,
        AF.Ln,
        AF.Relu,
        AF.Copy,
        AF.Identity,
        AF.Square,
        AF.Abs,
        AF.Sign,
        AF.MemsetZero,
        AF.Is_finite,
    }
    target = "natural_log_exp_and_others"

    def patched(module_arch):
        tables = orig(module_arch)
        if target not in tables:
            return tables
        out = {}
        for name, funcs in tables.items():
            out[name] = funcs if name == target else (funcs - keep)
        return out

    hw_specs.get_activation_tables = patched
    bacc.get_activation_tables = patched


def build_nc(
    repeat: int = 1, timing_trip: int | None = None, ablate: str | None = None
) -> bass.Bass:
    """timing_trip=T builds a timing variant: the whole computation runs in
    an on-device For_i loop T times, writing to internal DRAM scratch with a
    tiny external output, so real device time per iteration can be measured
    by wall-clock differencing of two trip counts (fixed host/transfer costs
    cancel; code size is constant)."""
    abl = set(ablate.split(",")) if ablate else set()
    if "nocs1bank" not in abl:
        abl.add("cs1bank")  # single-bank colsum accumulator (frees a PSUM
        # bank so the C-matmul pool triple-buffers)
    _force_single_act_table_set()
    nc = bacc.Bacc()
    xtT = nc.declare_dram_parameter("xtT", [B_LOC, F, N], fp16, isOutput=False)
    Wd = nc.declare_dram_parameter("W", [F, H], fp16, isOutput=False)
    bd = nc.declare_dram_parameter("b", [H, 1], f32, isOutput=False)
    if timing_trip is None:
        outd = nc.declare_dram_parameter("out", [B_LOC, N, N], bf16, isOutput=True)
    else:
        outd = nc.dram_tensor("oscratch", [B_LOC, N, N], bf16)
        tiny_out = nc.declare_dram_parameter("out", [2, 2], f32, isOutput=True)
    # host constants (engine APs must start at partition 0, so these cannot
    # be built with sliced memsets):
    #   cst col 0/1: per-partition scale/bias for the fused colsum Ln
    #   cuv cols 0:2 = Cu, 2:4 = Cv (f32r lhsT for the log-mix matmuls)
    cstd = nc.declare_dram_parameter("cst", [34, 4], f32, isOutput=False)
    cuvd = nc.declare_dram_parameter("cuv", [34, 34], mybir.dt.float32r, isOutput=False)


    with tile.TileContext(nc) as tc:
        with (
            tc.tile_pool(name="singles", bufs=1) as singles,
            tc.tile_pool(name="zpool", bufs=B_LOC) as zpool,
            tc.tile_pool(name="epool", bufs=6 + 2 * NT) as epool,
            tc.tile_pool(name="vpool", bufs=2) as vpool,
            tc.tile_pool(name="opool", bufs=8 if (ablate and "op8" in ablate) else 6) as opool,
            tc.tile_pool(name="apool", bufs=2, space="PSUM") as apool,
            tc.tile_pool(name="cpool", bufs=2 if (ablate and "nocs1bank" in ablate) else 3, space="PSUM") as cpool,
            tc.tile_pool(name="cspool", bufs=1, space="PSUM") as cspool,
        ):
            wsb = singles.tile([F, H], fp16)
            nc.gpsimd.dma_start(wsb[:], Wd[:, :])
            bsb = singles.tile([H, 1], f32)
            nc.gpsimd.dma_start(bsb[:], bd[:, :])
            ident = singles.tile([128, 128], bf16)
            make_identity(nc, ident[:])
            cm40 = singles.tile([128, 1], f32)
            nc.gpsimd.memset(cm40[:], KSHIFT)
            # per-partition [scale, bias] for the fused colsum Ln:
            # row 0: ln(0.5*cs0 + 1.0)   row 1: ln(1.0*cs1 + 0.0)
            cstsb = singles.tile([34, 4], f32)
            nc.gpsimd.dma_start(cstsb[:], cstd[:, :])
            cuvsb = singles.tile([34, 34], mybir.dt.float32r)
            nc.gpsimd.dma_start(cuvsb[:], cuvd[:, :])

            # ---- Z^T = relu(W^T @ xt^T + b) : [H, N] fp16, all batches
            # upfront (fills otherwise-idle engines during pipeline fill and
            # removes the Z chain from the batch-boundary critical path) ----
            zts = []
            for b in range(B_LOC):
                xtsb = zpool.tile([F, N], fp16, tag="xt")
                nc.sync.dma_start(xtsb[:], xtT[b])
                zpsum = apool.tile([H, N], f32, tag="ps")
                for j in range(NCK):
                    nc.tensor.matmul(
                        zpsum[:, j * CK : (j + 1) * CK],
                        wsb[:],
                        xtsb[:, j * CK : (j + 1) * CK],
                        start=True,
                        stop=True,
                    )
                zt = zpool.tile([H, N], fp16, tag="zt")
                # relu on DVE (ACT is the bottleneck engine): (Zpre + b) max 0
                nc.vector.tensor_scalar(
                    zt[:], zpsum[:], bsb[:], 0.0, ALU.add, ALU.max
                )
                # replicas at partitions 32/64/96: A_raw matmuls spread over
                # all four PE row groups (K=24 fits a 32-row group) so up to
                # four streams run concurrently in the array
                zreps = [zt]
                for g in (32, 64, 96):
                    ztg = zpool.tile([g + H, N], fp16, tag=f"zt{g}")
                    nc.sync.dma_start(ztg[g : g + H, :], zt[:])
                    zreps.append(ztg)
                zts.append(zreps)
            # r_buf col 1 stays 1.0 forever; col 0 is overwritten by the
            # recips each batch. One tile + one memset for the whole kernel.
            r_buf = singles.tile([128, NT, 2], bf16)
            nc.gpsimd.memset(r_buf[:], 1.0)

            def stats_tile(b, i, ztpair, rowsums, r_buf, cs, e_tiles):
                """A_raw matmul, exp(+rowsum), r_i, colsum accumulate.

                The colsum runs on PLAIN E (not E' = E + diag(rowsum)):
                degree = 1.5 + 0.5*colsum(r*E) and cs1 = colsum(E) = rowsum
                (by symmetry), so the diag fix stays off this critical
                chain -- it is emitted later, anywhere before the c-phase.

                The (chunk, tile-parity) pair selects one of the four PE row
                groups via Z replicas at partitions 0/32/64/96, so the two
                chunk matmuls of a tile AND adjacent tiles all overlap in
                the systolic array."""
                zreps = ztpair
                apsum = apool.tile([128, N], f32, tag="ps")
                for j in range(NCK):
                    g = 2 * (i % 2) + j  # 0..3
                    z = zreps[g]
                    base = (32 * g, 32 * g + H)
                    zs = z[base[0] : base[1], :] if g else z[:, :]
                    nc.tensor.matmul(
                        apsum[:, j * CK : (j + 1) * CK],
                        zs[:, i * 128 : (i + 1) * 128],
                        zs[:, j * CK : (j + 1) * CK],
                        start=True,
                        stop=True,
                        tile_position=(32 * g, 0),
                    )
                et = epool.tile([128, N], bf16, tag="E")
                nc.scalar.activation(
                    et[:],
                    apsum[:],
                    AF.Exp,
                    bias=cm40[:],
                    accum_out=rowsums[:, i, :],
                )
                if i % 2 == 1:
                    # one paired reciprocal per two tiles: a [128,1] recip
                    # costs ~455ns of DVE but [128,2] only ~150 (fixed
                    # dispatch dominates); the pair forces CSLAG=2
                    with nc.allow_low_precision("bf16 r for colsum lhsT"):
                        nc.vector.reciprocal(
                            r_buf[:, i - 1 : i + 1, 0:1], rowsums[:, i - 1 : i + 1, :]
                        )
                e_tiles.append(et)

            def colsum_tile(i, r_buf, cs, e_tiles):
                """Accumulate cs += [r_i, 1]^T @ E_i. Issued CSLAG tiles
                behind the exp so the in-order PE never stalls on the
                exp/recip semaphores. cs1bank layout: column-half j lands at
                partition rows 32j:32j+2 (tile_position col group j), so the
                accumulator fits one PSUM bank and the two chunks stream in
                different array column groups."""
                if "nocolsum" in abl:
                    return
                lhs = ident[:, 0:2] if "csnor" in abl else r_buf[:, i, :]
                for j in range(NCK):
                    if "cs1bank" in abl:
                        out_ap = cs[32 * j : 32 * j + 2, :]
                        tp = (0, 32 * j)
                    else:
                        out_ap = cs[:, j * CK : (j + 1) * CK]
                        tp = None
                    nc.tensor.matmul(
                        out_ap,
                        lhs,
                        e_tiles[i][:, j * CK : (j + 1) * CK],
                        start=(i == 0),
                        stop=(i == NT - 1),
                        skip_group_check=True,
                        tile_position=tp,
                    )

            def batch_tail(b, cs):
                """degree -> ds; u, v vectors (free layout).
                degree = 1 + 0.5*cs0 ; v = ds = exp(-0.5*ln(degree))
                u = 0.5*r*ds = exp(-ln(cs1) - 0.5*ln(degree))  (cs1 = 2*rowsum)
                Engine ops must be lane-aligned (partition base 0), so the
                log-domain row mixing runs on the PE (K=2 f32r matmuls
                against a tiny constant lhsT), never across partitions."""
                uv = vpool.tile([2, N], bf16, tag="uv")  # [u; v] (lhsT source)
                vu = vpool.tile([2, N], bf16, tag="vu")  # [v; u] (rhs source)
                if "cs1bank" in abl:
                    lls = vpool.tile([34, CK], mybir.dt.float32r, tag="lls")
                    nc.scalar.activation(
                        lls[:], cs[:, :], AF.Ln, bias=cstsb[:, 1:2], scale=cstsb[:, 0:1]
                    )
                else:
                    lls = vpool.tile([2, N], mybir.dt.float32r, tag="lls")
                    nc.scalar.activation(
                        lls[:],
                        cs[:, :],
                        AF.Ln,
                        bias=cstsb[0:2, 1:2],
                        scale=cstsb[0:2, 0:1],
                    )
                # single M=34 log-mix matmul per column-half: the
                # zero-padded coefficient lhsT (cuv cols 0:2 -> uv rows at
                # psum 0:2, cols 32:34 -> vu rows at psum 32:34) lets ONE
                # [34,N] exp produce both tail vectors (ACT free-dim cost:
                # 1061ns total instead of 2x)
                lmix = apool.tile([34, N], f32, tag="ps")
                for j in range(NCK):
                    if "cs1bank" in abl:
                        lhs_ap = cuvsb[32 * j : 32 * j + 2, :]
                        rhs_ap = lls[32 * j : 32 * j + 2, :]
                        tp = (32 * j, 0)
                    else:
                        lhs_ap = cuvsb[0:2, :]
                        rhs_ap = lls[:, j * CK : (j + 1) * CK]
                        tp = None
                    nc.tensor.matmul(
                        lmix[:, j * CK : (j + 1) * CK],
                        lhs_ap,
                        rhs_ap,
                        start=True,
                        stop=True,
                        tile_position=tp,
                    )
                uvvu = vpool.tile([34, N], bf16, tag="uvvu")
                nc.scalar.activation(uvvu[:], lmix[:], AF.Exp, bias=cstsb[:, 2:3])
                uv = uvvu[0:2, :]  # [u; v] at base 0 (even-tile lhsT)
                # vu at base 0 (even-tile rhs) via DVE copy (658ns, no DMA
                # fixed cost); uv at base 32 (odd-tile lhsT) via scalar-HWDGE
                # DMA (first odd c-tile is one tile later: latency hidden).
                # Odd-tile rhs is uvvu[32:34] natively.
                vu = vpool.tile([2, N], bf16, tag="vu")
                nc.vector.tensor_copy(vu[:], uvvu[32:34, :])
                uv32 = vpool.tile([34, N], bf16, tag="uv32")
                nc.scalar.dma_start(uv32[32:34, :], uvvu[0:2, :])
                return uv, vu, uv32, uvvu[32:34, :]

            def diag_patch(i, rowsums, e_tiles):
                """E'[n,n] = E[n,n] + rowsum[n] (folds "+I" into the final
                multiply; one cheap TS+TT pair per tile, ~420ns of DVE).
                Measured faster on DVE than Pool with the triple-buffered
                cpool (Pool ops hold the shared DVE/Pool SBUF port).
                Ordered after the colsum's plain-E read by Tile's WAR
                tracking; needed before the c-phase."""
                et = e_tiles[i]
                # fused: E'_diag = (ident * rowsum[p]) + E_diag -- one DVE op
                # (scalar_tensor_tensor) instead of a TS+TT pair
                nc.vector.scalar_tensor_tensor(
                    et[:, i * 128 : (i + 1) * 128],
                    ident[:],
                    rowsums[:, i, :],
                    et[:, i * 128 : (i + 1) * 128],
                    ALU.mult,
                    ALU.add,
                )

            def c_tile(b, i, uv, vu, uv64, vu64, e_tiles, last_batch):
                """C = u v^T + v u^T (K=2 matmul), out = E' * C, DMA out."""
                osb = opool.tile([128, N], bf16, tag="o")
                if i % 2 == 0 or "norep" in abl:
                    uvs, vus = uv, vu
                else:
                    uvs, vus = uv64[32:34, :], vu64
                for j in range(NCK):
                    cps = cpool.tile([128, CK], f32, tag="c")
                    nc.tensor.matmul(
                        cps[:],
                        uvs[:, i * 128 : (i + 1) * 128],
                        vus[:, j * CK : (j + 1) * CK],
                        start=True,
                        stop=True,
                    )
                    if last_batch and i < (6 if "drain6" in abl else 5) and "nodrainsplit" not in abl:
                        # drain phase: ACT is idle -- route the PSUM exit
                        # through an ACT Copy so the multiply runs at the
                        # 2x both-SBUF TT rate (DVE 824ns/tile vs 1408)
                        csb = opool.tile([128, CK], bf16, tag="csb")
                        nc.scalar.activation(csb[:], cps[:], AF.Copy)
                        nc.vector.tensor_tensor(
                            osb[:, j * CK : (j + 1) * CK],
                            e_tiles[i][:, j * CK : (j + 1) * CK],
                            csb[:],
                            ALU.mult,
                        )
                    else:
                        nc.vector.tensor_tensor(
                            osb[:, j * CK : (j + 1) * CK],
                            e_tiles[i][:, j * CK : (j + 1) * CK],
                            cps[:],
                            ALU.mult,
                        )
                if "nodma" not in abl:
                    nc.sync.dma_start(outd[b, i * 128 : (i + 1) * 128, :], osb[:])


            # software pipeline: batch b's stats tiles interleave with batch
            # b-1's output tiles so PE/DVE/DMA trail ACT by one phase.
            # At each batch crossing, the next batch's first two stats tiles
            # are emitted BEFORE the trailing colsums + tail (keeps ACT fed
            # while the cs/Ln chain resolves), and the previous batch's last
            # two c-tiles land inside the crossing (keeps DVE fed during the
            # tail's Ln/lmix/exp latency).
            def emit_pipeline(last_rep):
                if "oldpipe" not in abl:
                    emit_pipeline_xing(last_rep)
                    return
                prev = None
                for b in range(B_LOC):
                    rowsums = vpool.tile([128, NT, 1], f32, tag="rowsums")
                    # cs[0,m] = sum_n r[n] E[n,m] = w; cs[1,m] = rowsum[m]
                    # (cs1bank: column-half j lives at partition rows 32j:32j+2
                    #  so the accumulator fits one PSUM bank)
                    if "cs1bank" in abl:
                        cs = cspool.tile([34, CK], f32, tag="cs")
                    else:
                        cs = cspool.tile([2, N], f32, tag="cs")
                    if "nocolsum" in abl:
                        nc.vector.memset(cs[:], 1.0)
                    e_tiles = []
                    if "csburst" in abl:
                        # colsums in two contiguous bursts: the K=128
                        # full-array matmuls drain the 32-row-group streams
                        # (A_raw/C) once per burst instead of per pair
                        for i in range(NT):
                            stats_tile(b, i, zts[b], rowsums, r_buf, cs, e_tiles)
                            if i == NT - 3:
                                for t in range(NT // 2):
                                    colsum_tile(t, r_buf, cs, e_tiles)
                                    if "nodiag" not in abl:
                                        diag_patch(t, rowsums, e_tiles)
                            if prev is not None and "statsonly" not in abl:
                                c_tile(prev[0], i, *prev[1:], False)
                        for t in range(NT // 2, NT):
                            colsum_tile(t, r_buf, cs, e_tiles)
                            if "nodiag" not in abl:
                                diag_patch(t, rowsums, e_tiles)
                    else:
                        CSLAG = 2 if "cslag2" in abl else (4 if "cslag4" in abl else 3)
                        for i in range(NT):
                            stats_tile(b, i, zts[b], rowsums, r_buf, cs, e_tiles)
                            if "csfirst" in abl:
                                if i >= CSLAG:
                                    colsum_tile(i - CSLAG, r_buf, cs, e_tiles)
                                    if "nodiag" not in abl:
                                        diag_patch(i - CSLAG, rowsums, e_tiles)
                                if prev is not None and "statsonly" not in abl:
                                    c_tile(prev[0], i, *prev[1:], False)
                            else:
                                if prev is not None and "statsonly" not in abl:
                                    c_tile(prev[0], i, *prev[1:], False)
                                if i >= CSLAG:
                                    colsum_tile(i - CSLAG, r_buf, cs, e_tiles)
                                    if "nodiag" not in abl:
                                        diag_patch(i - CSLAG, rowsums, e_tiles)
                        for i in range(NT - CSLAG, NT):
                            colsum_tile(i, r_buf, cs, e_tiles)
                            if "nodiag" not in abl:
                                diag_patch(i, rowsums, e_tiles)
                    uv, vu, uv64, vu64 = batch_tail(b, cs)
                    prev = (b, uv, vu, uv64, vu64, e_tiles)
                if "statsonly" not in abl:
                    for i in range(NT):
                        c_tile(prev[0], i, *prev[1:], last_rep)

            def emit_pipeline_xing(last_rep):
                PRE = 2  # next-batch stats tiles pre-emitted at the crossing
                CSLAG = 3
                states = {}

                def new_state(b):
                    rowsums = vpool.tile([128, NT, 1], f32, tag="rowsums")
                    if "cs1bank" in abl:
                        cs = cspool.tile([34, CK], f32, tag="cs")
                    else:
                        cs = cspool.tile([2, N], f32, tag="cs")
                    if "nocolsum" in abl:
                        nc.vector.memset(cs[:], 1.0)
                    states[b] = (rowsums, cs, [])

                def stats(b, i):
                    rowsums, cs, e_tiles = states[b]
                    stats_tile(b, i, zts[b], rowsums, r_buf, cs, e_tiles)

                def cs_diag(b, t):
                    rowsums, cs, e_tiles = states[b]
                    colsum_tile(t, r_buf, cs, e_tiles)
                    if "nodiag" not in abl:
                        diag_patch(t, rowsums, e_tiles)

                prev = None
                for b in range(B_LOC):
                    if b == 0:
                        new_state(0)
                    for i in range(0 if b == 0 else PRE, NT):
                        stats(b, i)
                        if i >= CSLAG:
                            cs_diag(b, i - CSLAG)
                        if prev is not None and "statsonly" not in abl:
                            c_tile(prev[0], i - PRE, *prev[1:], False)
                    # crossing: b+1's first stats tiles between b's trailing
                    # colsums so ACT never starves on the cs->Ln chain
                    if b + 1 < B_LOC:
                        new_state(b + 1)
                        stats(b + 1, 0)
                        cs_diag(b, NT - CSLAG)
                        stats(b + 1, 1)
                        for t in range(NT - CSLAG + 1, NT):
                            cs_diag(b, t)
                    else:
                        for t in range(NT - CSLAG, NT):
                            cs_diag(b, t)
                    if prev is not None and "statsonly" not in abl:
                        for i in range(NT - PRE, NT):
                            c_tile(prev[0], i, *prev[1:], False)
                    rowsums, cs, e_tiles = states[b]
                    uv, vu, uv64, vu64 = batch_tail(b, cs)
                    prev = (b, uv, vu, uv64, vu64, e_tiles)
                if "statsonly" not in abl:
                    for i in range(NT):
                        c_tile(prev[0], i, *prev[1:], last_rep)

            if timing_trip is None:
                for rep in range(repeat):
                    emit_pipeline(rep == repeat - 1)
            else:
                with tc.For_i(0, timing_trip, 1):
                    emit_pipeline(False)
                tiny = singles.tile([2, 2], f32)
                nc.gpsimd.memset(tiny[:], 1.0)
                nc.sync.dma_start(tiny_out[:, :], tiny[:])

    nc.finalize()
    return nc


_NC_CACHE = None


def _get_nc() -> bass.Bass:
    global _NC_CACHE
    if _NC_CACHE is None:
        _NC_CACHE = build_nc()
    return _NC_CACHE


def _make_in_maps(xt: np.ndarray, W: np.ndarray, b: np.ndarray):
    xtT = np.ascontiguousarray(np.asarray(xt).transpose(0, 2, 1)).astype(np.float16)
    Wh = np.ascontiguousarray(np.asarray(W)).astype(np.float16)
    bh = np.ascontiguousarray(np.asarray(b)).reshape(H, 1).astype(np.float32)
    # cst cols: [Ln scale, Ln bias, uv-exp bias, vu-exp bias]
    # degree = 1.5 + 0.5*cs0 (plain-E colsum); cs1 = rowsum
    # u = exp(-0.5*ldeg - ln rs + ln 0.5), v = exp(-0.5*ldeg)
    ln_half = float(np.log(0.5))
    cst2 = np.array(
        [[0.5, 1.5, ln_half, 0.0], [1.0, 0.0, 0.0, ln_half]], dtype=np.float32
    )
    cuv2 = np.array(
        [[-0.5, -0.5, -0.5, -0.5], [-1.0, 0.0, 0.0, -1.0]], dtype=np.float32
    )
    # [34,4]: rows 32:34 replicate rows 0:2 (cs1bank column-half 1); pad rows
    # use scale 0 / bias 1 so the Ln of junk partitions stays finite
    cst = np.zeros((34, 4), dtype=np.float32)
    cst[:, 1] = 1.0
    cst[0:2] = cst2
    cst[32:34] = cst2
    # combined-exp bias col 2: rows 0:2 = uv bias [ln1/2, 0], rows 32:34 =
    # vu bias [0, ln1/2]; pad rows 0 (exp(0)=1, finite, never read)
    cst[:, 2] = 0.0
    cst[0, 2] = ln_half
    cst[33, 2] = ln_half
    # lmix coefficients: out row 0 = u-pre (-0.5*ldeg - lnrs), row 1 =
    # v-pre (-0.5*ldeg); rows 32/33 = v-pre/u-pre ([v; u] order)
    cuvL = np.zeros((2, 34), dtype=np.float32)
    cuvL[:, 0] = (-0.5, -1.0)
    cuvL[:, 1] = (-0.5, 0.0)
    cuvL[:, 32] = (-0.5, 0.0)
    cuvL[:, 33] = (-0.5, -1.0)
    cuv = np.zeros((34, 34), dtype=np.float32)
    cuv[0:2] = cuvL
    cuv[32:34] = cuvL
    return [
        {
            "xtT": xtT[B_LOC * k : B_LOC * (k + 1)],
            "W": Wh,
            "b": bh,
            "cst": cst,
            "cuv": cuv,
        }
        for k in range(N_CORES)
    ]


def run(xt, W, b, trace: bool = False):
    """Run on 8 NeuronCores; returns (out, BassKernelResults)."""
    res = run_bass_kernel_spmd(
        _get_nc(), _make_in_maps(xt, W, b), core_ids=list(range(N_CORES)), trace=trace
    )
    out = np.concatenate(
        [np.asarray(res.results[k]["out"]) for k in range(N_CORES)], axis=0
    )
    return out.astype(np.float32, copy=False), res


def kernel(xt: np.ndarray, W: np.ndarray, b: np.ndarray) -> np.ndarray:
    out, _ = run(xt, W, b, trace=False)
    return out

